# revision 24
# baseline (speedup 1.0000x reference)
"""Sparse multi-head attention (nn_MultiHeadAttention_44332652429419) on 8 trn2 cores.

Strategy (tensor-parallel over H=16 heads, 2 heads per core):
  Host: compose the two stacked linear layers (q/k/v_proj followed by
        MultiheadAttention in_proj) into one weight per tensor; build the
        dense multiplicative mask exp(additive_mask) transposed; transpose x.
  Device (per core, SPMD with per-core weight slices):
    q2T/k2T/v2T = W_c @ x.T + b_c           [128, 3072] (2 heads x 64 dims)
    scoresT[mk,nq] = k2T_h.T-slice @ q2T_h  (K=64, two heads row-packed in PE)
    P = exp(scoresT * 1/8) * maskT          (ACT exp from PSUM, DVE multiply)
    outT_aug = [v_h | 1].T @ P              (rowsum via ones-augmented V)
    attnT = outT[:64] / outT[64]            (DVE recip + partition broadcast)
    ypart = attnT.T-slices @ woT_c          (partial out_proj, K=128)
  Host: y = sum_c ypart_c + bo
"""
import os
import sys

sys.path.insert(0, "/opt/trn_rl_repo")

import numpy as np
from contextlib import ExitStack

import concourse.bass as bass
import concourse.bacc as bacc
import concourse.mybir as mybir
import concourse.tile as tile
from concourse.bass_utils import run_bass_kernel_spmd
from concourse.masks import make_identity

F32 = mybir.dt.float32
F32R = mybir.dt.float32r
BF16 = mybir.dt.bfloat16
AF = mybir.ActivationFunctionType
ALU = mybir.AluOpType

N = 3072
IN_F = 1024
OUT_F = 1024
H = 16
D = 64
NCORES = 8
HPC = H // NCORES            # heads per core = 2
CW = HPC * D                 # per-core width = 128
P = 128
NT = N // P                  # 24 node tiles
KT = IN_F // P               # 8 contraction tiles
SQ = 1024                    # query strip width (phase B)
NSQ = N // SQ                # 3 strips
SP = 512                     # proj strip width (phase A)
NSP = N // SP                # 6 strips
SCALE = 1.0 / 8.0            # 1/sqrt(D)

MASK_DT = F32R               # additive mask, pre-scaled by 1/SCALE
MASK_ALL_PE = True           # apply mask via PE identity-add for all tiles


def build_program():
    nc = bacc.Bacc()
    xT = nc.declare_dram_parameter("xT", [IN_F, N], F32R, isOutput=False)
    # additive mask (pre-scaled by 1/SCALE) for even key tiles, multiplicative
    # exp-mask for odd key tiles — hybrid PE/DVE mask application
    maskA = nc.declare_dram_parameter("maskA", [N // 2, N], F32R, isOutput=False)
    maskM = nc.declare_dram_parameter("maskM", [N // 2, N],
                                     F32R if MASK_ALL_PE else F32, isOutput=False)
    wqT = nc.declare_dram_parameter("wqT", [IN_F, CW], F32R, isOutput=False)
    wkT = nc.declare_dram_parameter("wkT", [IN_F, CW], F32R, isOutput=False)
    wvT = nc.declare_dram_parameter("wvT", [IN_F, CW], F32R, isOutput=False)
    bq = nc.declare_dram_parameter("bq", [CW], F32, isOutput=False)
    bk = nc.declare_dram_parameter("bk", [CW], F32, isOutput=False)
    bv = nc.declare_dram_parameter("bv", [CW], F32, isOutput=False)
    woT = nc.declare_dram_parameter("woT", [CW, OUT_F], F32R, isOutput=False)
    ypart = nc.declare_dram_parameter("ypart", [N, OUT_F], F32, isOutput=True)

    with tile.TileContext(nc) as tc, ExitStack() as ctx:
        cst = ctx.enter_context(tc.tile_pool(name="cst", bufs=1))
        lp = ctx.enter_context(tc.tile_pool(name="lp", bufs=2))       # xs/v2Ts
        wp = ctx.enter_context(tc.tile_pool(name="wp", bufs=3))       # loop tiles
        ep = ctx.enter_context(tc.tile_pool(name="ep", bufs=2))       # epilogue
        pp = ctx.enter_context(tc.tile_pool(name="pp", bufs=2, space="PSUM"))
        pso = ctx.enter_context(tc.tile_pool(name="pso", bufs=1, space="PSUM"))

        ident = cst.tile([P, P], F32)
        make_identity(nc, ident)
        identR = cst.tile([P, P], F32R)
        nc.vector.tensor_copy(identR[:], ident[:])

        # per-512-strip persistent tensors (fine-grained deps let phase B
        # start while projections still run)
        q2s = [cst.tile([P, SP], F32R, tag=f"q2s{s}", name=f"q2s{s}")
               for s in range(NSP)]
        # k2z[h][s]: only rows h*D..h*D+63 live, rest zero — score matmuls
        # contract over full K=128 (keeps the PE HAM activity monitor warm)
        k2zs = [[cst.tile([P, SP], F32R, tag=f"k2z{h}_{s}", name=f"k2z{h}_{s}")
                 for s in range(NSP)] for h in range(HPC)]
        attn_t = [cst.tile([P, P], F32R, tag=f"attn{t}", name=f"attn{t}")
                  for t in range(NT)]
        vaug = [cst.tile([P, NT, D + 1], F32R, tag=f"vaug{h}", name=f"vaug{h}")
                for h in range(HPC)]
        ones_col = cst.tile([P, 1], F32)
        nc.vector.memset(ones_col[:], 1.0)
        zero_col = cst.tile([P, 1], F32)
        nc.vector.memset(zero_col[:], 0.0)
        for h in range(HPC):
            nc.vector.tensor_copy(vaug[h][:, :, D:D + 1],
                                  ones_col[:, 0:1, None].to_broadcast([P, NT, 1]))
            osl = slice((1 - h) * D, (2 - h) * D)   # the dead half of k2z[h]
            for s in range(NSP):
                nc.vector.tensor_copy(k2zs[h][s][osl, :],
                                      zero_col[osl, 0:1].to_broadcast([D, SP]))

        # weights
        wq_sb = cst.tile([P, KT, CW], F32R)
        nc.sync.dma_start(wq_sb[:], wqT.rearrange("(k p) m -> p k m", p=P))
        wk_sb = cst.tile([P, KT, CW], F32R)
        nc.sync.dma_start(wk_sb[:], wkT.rearrange("(k p) m -> p k m", p=P))
        wv_sb = cst.tile([P, KT, CW], F32R)
        nc.sync.dma_start(wv_sb[:], wvT.rearrange("(k p) m -> p k m", p=P))
        wo_sb = cst.tile([P, OUT_F], F32R)
        nc.sync.dma_start(wo_sb[:], woT[:])
        bq_sb = cst.tile([P, 1], F32)
        nc.sync.dma_start(bq_sb[:], bq[:, None])
        bk_sb = cst.tile([P, 1], F32)
        nc.sync.dma_start(bk_sb[:], bk[:, None])
        bv_sb = cst.tile([P, 1], F32)
        nc.sync.dma_start(bv_sb[:], bv[:, None])

        # ---- Phase A: projections (per 512-node strip) ----
        for s in range(NSP):
            xs = lp.tile([P, KT, SP], F32R, tag="xs")
            nc.sync.dma_start(
                xs[:], xT.rearrange("(k p) n -> p k n", p=P)[:, :, s * SP:(s + 1) * SP])
            ps = pp.tile([P, SQ], F32, tag="ps_s", name="ps_q")
            for k in range(KT):
                nc.tensor.matmul(ps[:, 0:SP], wq_sb[:, k, :], xs[:, k, :],
                                 start=(k == 0), stop=(k == KT - 1))
            nc.vector.tensor_scalar_add(q2s[s][:], ps[:, 0:SP], bq_sb[:, 0:1])
            ps = pp.tile([P, SQ], F32, tag="ps_s", name="ps_k")
            for k in range(KT):
                nc.tensor.matmul(ps[:, 0:SP], wk_sb[:, k, :], xs[:, k, :],
                                 start=(k == 0), stop=(k == KT - 1))
            for h in range(HPC):
                hsl = slice(h * D, (h + 1) * D)
                nc.vector.tensor_scalar_add(k2zs[h][s][hsl, :], ps[hsl, 0:SP],
                                            bk_sb[hsl, 0:1])
            # v: project then transpose into vaug
            ps = pp.tile([P, SQ], F32, tag="ps_s", name="ps_v")
            for k in range(KT):
                nc.tensor.matmul(ps[:, 0:SP], wv_sb[:, k, :], xs[:, k, :],
                                 start=(k == 0), stop=(k == KT - 1))
            v2Ts = lp.tile([P, SP], F32, tag="v2Ts")
            nc.vector.tensor_scalar_add(v2Ts[:], ps[:, 0:SP], bv_sb[:, 0:1])
            for b in range(SP // P):
                t = s * (SP // P) + b
                ps_t = pp.tile([P, SQ], F32, tag="ps_s", name="ps_t")
                nc.tensor.transpose(ps_t[:, 0:P], v2Ts[:, b * P:(b + 1) * P],
                                    ident[:])
                for h in range(HPC):
                    nc.vector.tensor_copy(vaug[h][:, t, 0:D],
                                          ps_t[:, h * D:h * D + D])

        # ---- Phase B: attention + fused out_proj ----
        for sq in range(NSQ):
            ps_o = [pso.tile([D + 1, SQ], F32, tag=f"ps_o{h}", name=f"ps_o{h}")
                    for h in range(HPC)]
            for mk in range(NT):
                use_pe = (mk % 2 == 0) or MASK_ALL_PE
                if mk % 2 == 0:
                    mt = wp.tile([P, SQ], F32R, tag="mta")
                    nc.sync.dma_start(
                        mt[:], maskA[(mk // 2) * P:(mk // 2 + 1) * P,
                                     sq * SQ:(sq + 1) * SQ])
                else:
                    mt = wp.tile([P, SQ], F32R if MASK_ALL_PE else F32, tag="mtm")
                    nc.sync.dma_start(
                        mt[:], maskM[(mk // 2) * P:(mk // 2 + 1) * P,
                                     sq * SQ:(sq + 1) * SQ])
                for h in range(HPC):
                    ps_s = pp.tile([P, SQ], F32, tag="ps_s", name="ps_s")
                    for half in range(SQ // SP):
                        fsl = slice(half * SP, (half + 1) * SP)
                        nc.tensor.matmul(
                            ps_s[:, fsl],
                            k2zs[h][mk // 4][:, (mk % 4) * P:(mk % 4 + 1) * P],
                            q2s[sq * (SQ // SP) + half][:],
                            start=True, stop=not use_pe,
                        )
                        if use_pe:
                            nc.tensor.matmul(
                                ps_s[:, fsl], identR[:], mt[:, fsl],
                                start=False, stop=True,
                            )
                    p = wp.tile([P, SQ], F32R, tag="p")
                    nc.scalar.activation(p[:], ps_s[:], AF.Exp, scale=SCALE)
                    if not use_pe:
                        pm = wp.tile([P, SQ], F32R, tag="pm")
                        nc.vector.tensor_tensor(pm[:], p[:], mt[:], ALU.mult)
                        p = pm
                    for half in range(SQ // SP):
                        fsl = slice(half * SP, (half + 1) * SP)
                        nc.tensor.matmul(
                            ps_o[h][:, fsl],
                            vaug[h][:, mk, :],
                            p[:, fsl],
                            start=(mk == 0), stop=(mk == NT - 1),
                        )
            bcs = []
            for h in range(HPC):
                zrow = ep.tile([1, SQ], F32, tag="zrow", name=f"zrow{h}")
                nc.vector.tensor_copy(zrow[:], ps_o[h][D:D + 1, :])
                recip = ep.tile([1, SQ], F32, tag="recip", name=f"recip{h}")
                nc.vector.reciprocal_approx_fast(recip[:], zrow[:])
                bc = ep.tile([D, SQ], F32, tag=f"bc{h}", name=f"bc{h}")
                nc.gpsimd.partition_broadcast(bc[:], recip[:])
                bcs.append(bc)
            # normalize per node tile so out_proj starts early
            for b in range(SQ // P):
                t = sq * (SQ // P) + b
                for h in range(HPC):
                    nc.vector.tensor_tensor(
                        attn_t[t][h * D:(h + 1) * D, :],
                        ps_o[h][0:D, b * P:(b + 1) * P],
                        bcs[h][:, b * P:(b + 1) * P], ALU.mult)
                ps_y = pp.tile([P, SQ], F32, tag="ps_s", name="ps_y")
                for f in range(OUT_F // 512):
                    nc.tensor.matmul(ps_y[:, f * 512:(f + 1) * 512],
                                     attn_t[t][:, :],
                                     wo_sb[:, f * 512:(f + 1) * 512],
                                     start=True, stop=True)
                ys = wp.tile([P, OUT_F], F32, tag="ys")
                if b % 2 == 0:
                    nc.vector.tensor_copy(ys[:], ps_y[:])
                else:
                    nc.scalar.copy(ys[:], ps_y[:])
                nc.sync.dma_start(ypart[t * P:(t + 1) * P, :], ys[:])

    nc.compile()
    return nc


_PROGRAM = None
LAST_RESULTS = None


def _get_program():
    global _PROGRAM
    if _PROGRAM is None:
        _PROGRAM = build_program()
    return _PROGRAM


def _softplus(x):
    x = np.asarray(x, np.float32)
    return np.logaddexp(0.0, x).astype(np.float32)


def host_prep(inputs):
    x = np.asarray(inputs["x"], np.float32)
    edge_index = np.asarray(inputs["edge_index"])
    edge_type = np.asarray(inputs["edge_type"])
    etw = np.asarray(inputs["edge_type_weights"], np.float32)

    def f32(k):
        return np.asarray(inputs[k], np.float32)

    # compose the two linear layers: q2 = x @ (wiq@wq).T + (wiq@bq + biq)
    WQ = f32("wiq") @ f32("wq")
    bQ = f32("wiq") @ f32("bq") + f32("biq")
    WK = f32("wik") @ f32("wk")
    bK = f32("wik") @ f32("bk") + f32("bik")
    WV = f32("wiv") @ f32("wv")
    bV = f32("wiv") @ f32("bv") + f32("biv")
    wo = f32("wo")
    bo = f32("bo")

    # multiplicative mask, transposed: maskT[m, n] = exp(add_mask[n, m])
    w = _softplus(etw)
    NEG = np.float32(-8e30)
    M = np.full((N, N), NEG, dtype=np.float32)
    src, dst = edge_index[0], edge_index[1]
    wv8 = (w * np.float32(1.0 / SCALE)).astype(np.float32)
    M[src, dst] = wv8[edge_type - 1]           # last write wins, like jax .at[].set
    diag = np.diagonal(M).copy()
    didx = np.arange(N)
    M[didx, didx] = np.where(diag == NEG, wv8[3], diag)
    MT = np.ascontiguousarray(M.T)             # [key m, query n], additive * 8
    # even key tiles use the additive form on the PE, odd tiles the
    # multiplicative exp-form on the DVE
    MT4 = MT.reshape(NT, P, N)
    maskA = np.ascontiguousarray(MT4[0::2].reshape(N // 2, N))
    if MASK_ALL_PE:
        maskM = np.ascontiguousarray(MT4[1::2].reshape(N // 2, N))
    else:
        maskM = np.exp(MT4[1::2].reshape(N // 2, N).astype(np.float64)
                       * np.float64(SCALE)).astype(np.float32)

    xT = np.ascontiguousarray(x.T)

    in_maps = []
    for c in range(NCORES):
        rs = slice(c * CW, (c + 1) * CW)
        in_maps.append({
            "xT": xT,
            "maskA": maskA,
            "maskM": maskM,
            "wqT": np.ascontiguousarray(WQ[rs].T),
            "wkT": np.ascontiguousarray(WK[rs].T),
            "wvT": np.ascontiguousarray(WV[rs].T),
            "bq": np.ascontiguousarray(bQ[rs]),
            "bk": np.ascontiguousarray(bK[rs]),
            "bv": np.ascontiguousarray(bV[rs]),
            "woT": np.ascontiguousarray(wo[:, rs].T),
        })
    return in_maps, bo


def kernel(**inputs) -> np.ndarray:
    global LAST_RESULTS
    in_maps, bo = host_prep(inputs)
    nc = _get_program()
    trace = bool(os.environ.get("KERNEL_TRACE"))
    res = run_bass_kernel_spmd(nc, in_maps, list(range(NCORES)), trace=trace)
    LAST_RESULTS = res
    y = bo[None, :].astype(np.float32).repeat(N, axis=0)
    for c in range(NCORES):
        y += res.results[c]["ypart"]
    return y


# revision 25
# speedup vs baseline: 1.0443x; 1.0443x over previous
"""Sparse multi-head attention (nn_MultiHeadAttention_44332652429419) on 8 trn2 cores.

Strategy (tensor-parallel over H=16 heads, 2 heads per core):
  Host: compose the two stacked linear layers (q/k/v_proj followed by
        MultiheadAttention in_proj) into one weight per tensor; build the
        dense multiplicative mask exp(additive_mask) transposed; transpose x.
  Device (per core, SPMD with per-core weight slices):
    q2T/k2T/v2T = W_c @ x.T + b_c           [128, 3072] (2 heads x 64 dims)
    scoresT[mk,nq] = k2T_h.T-slice @ q2T_h  (K=64, two heads row-packed in PE)
    P = exp(scoresT * 1/8) * maskT          (ACT exp from PSUM, DVE multiply)
    outT_aug = [v_h | 1].T @ P              (rowsum via ones-augmented V)
    attnT = outT[:64] / outT[64]            (DVE recip + partition broadcast)
    ypart = attnT.T-slices @ woT_c          (partial out_proj, K=128)
  Host: y = sum_c ypart_c + bo
"""
import os
import sys

sys.path.insert(0, "/opt/trn_rl_repo")

import numpy as np
from contextlib import ExitStack

import concourse.bass as bass
import concourse.bacc as bacc
import concourse.mybir as mybir
import concourse.tile as tile
from concourse.bass_utils import run_bass_kernel_spmd
from concourse.masks import make_identity

F32 = mybir.dt.float32
F32R = mybir.dt.float32r
BF16 = mybir.dt.bfloat16
AF = mybir.ActivationFunctionType
ALU = mybir.AluOpType

N = 3072
IN_F = 1024
OUT_F = 1024
H = 16
D = 64
NCORES = 8
HPC = H // NCORES            # heads per core = 2
CW = HPC * D                 # per-core width = 128
P = 128
NT = N // P                  # 24 node tiles
KT = IN_F // P               # 8 contraction tiles
SQ = 1024                    # query strip width (phase B)
NSQ = N // SQ                # 3 strips
SP = 512                     # proj strip width (phase A)
NSP = N // SP                # 6 strips
SCALE = 1.0 / 8.0            # 1/sqrt(D)

MASK_DT = F32R               # additive mask, pre-scaled by 1/SCALE
MASK_ALL_PE = False          # apply mask via PE identity-add for all tiles


def build_program():
    nc = bacc.Bacc()
    xT = nc.declare_dram_parameter("xT", [IN_F, N], F32R, isOutput=False)
    # additive mask (pre-scaled by 1/SCALE) for even key tiles, multiplicative
    # exp-mask for odd key tiles — hybrid PE/DVE mask application
    maskA = nc.declare_dram_parameter("maskA", [N // 2, N], F32R, isOutput=False)
    maskM = nc.declare_dram_parameter("maskM", [N // 2, N],
                                     F32R if MASK_ALL_PE else F32, isOutput=False)
    wqT = nc.declare_dram_parameter("wqT", [IN_F, CW], F32R, isOutput=False)
    wkT = nc.declare_dram_parameter("wkT", [IN_F, CW], F32R, isOutput=False)
    wvT = nc.declare_dram_parameter("wvT", [IN_F, CW], F32R, isOutput=False)
    bq = nc.declare_dram_parameter("bq", [CW], F32, isOutput=False)
    bk = nc.declare_dram_parameter("bk", [CW], F32, isOutput=False)
    bv = nc.declare_dram_parameter("bv", [CW], F32, isOutput=False)
    woT = nc.declare_dram_parameter("woT", [CW, OUT_F], F32R, isOutput=False)
    ypart = nc.declare_dram_parameter("ypart", [N, OUT_F], F32, isOutput=True)

    with tile.TileContext(nc) as tc, ExitStack() as ctx:
        cst = ctx.enter_context(tc.tile_pool(name="cst", bufs=1))
        lp = ctx.enter_context(tc.tile_pool(name="lp", bufs=2))       # xs/v2Ts
        wp = ctx.enter_context(tc.tile_pool(name="wp", bufs=3))       # loop tiles
        ep = ctx.enter_context(tc.tile_pool(name="ep", bufs=2))       # epilogue
        pp = ctx.enter_context(tc.tile_pool(name="pp", bufs=2, space="PSUM"))
        pso = ctx.enter_context(tc.tile_pool(name="pso", bufs=1, space="PSUM"))

        ident = cst.tile([P, P], F32)
        make_identity(nc, ident)
        identR = cst.tile([P, P], F32R)
        nc.vector.tensor_copy(identR[:], ident[:])

        # per-512-strip persistent tensors (fine-grained deps let phase B
        # start while projections still run)
        q2s = [cst.tile([P, SP], F32R, tag=f"q2s{s}", name=f"q2s{s}")
               for s in range(NSP)]
        # k2z[h][s]: only rows h*D..h*D+63 live, rest zero — score matmuls
        # contract over full K=128 (keeps the PE HAM activity monitor warm)
        k2zs = [[cst.tile([P, SP], F32R, tag=f"k2z{h}_{s}", name=f"k2z{h}_{s}")
                 for s in range(NSP)] for h in range(HPC)]
        attn_t = [cst.tile([P, P], F32R, tag=f"attn{t}", name=f"attn{t}")
                  for t in range(NT)]
        vaug = [cst.tile([P, NT, D + 1], F32R, tag=f"vaug{h}", name=f"vaug{h}")
                for h in range(HPC)]
        ones_col = cst.tile([P, 1], F32)
        nc.vector.memset(ones_col[:], 1.0)
        zero_col = cst.tile([P, 1], F32)
        nc.vector.memset(zero_col[:], 0.0)
        for h in range(HPC):
            nc.vector.tensor_copy(vaug[h][:, :, D:D + 1],
                                  ones_col[:, 0:1, None].to_broadcast([P, NT, 1]))
            osl = slice((1 - h) * D, (2 - h) * D)   # the dead half of k2z[h]
            for s in range(NSP):
                nc.vector.tensor_copy(k2zs[h][s][osl, :],
                                      zero_col[osl, 0:1].to_broadcast([D, SP]))

        # weights
        wq_sb = cst.tile([P, KT, CW], F32R)
        nc.sync.dma_start(wq_sb[:], wqT.rearrange("(k p) m -> p k m", p=P))
        wk_sb = cst.tile([P, KT, CW], F32R)
        nc.sync.dma_start(wk_sb[:], wkT.rearrange("(k p) m -> p k m", p=P))
        wv_sb = cst.tile([P, KT, CW], F32R)
        nc.sync.dma_start(wv_sb[:], wvT.rearrange("(k p) m -> p k m", p=P))
        wo_sb = cst.tile([P, OUT_F], F32R)
        nc.sync.dma_start(wo_sb[:], woT[:])
        bq_sb = cst.tile([P, 1], F32)
        nc.sync.dma_start(bq_sb[:], bq[:, None])
        bk_sb = cst.tile([P, 1], F32)
        nc.sync.dma_start(bk_sb[:], bk[:, None])
        bv_sb = cst.tile([P, 1], F32)
        nc.sync.dma_start(bv_sb[:], bv[:, None])

        # ---- emission helpers ----
        def emit_proj_strip(s):
            xs = lp.tile([P, KT, SP], F32R, tag="xs", name="xs")
            nc.sync.dma_start(
                xs[:], xT.rearrange("(k p) n -> p k n", p=P)[:, :, s * SP:(s + 1) * SP])
            ps = pp.tile([P, SQ], F32, tag="ps_s", name="ps_q")
            for k in range(KT):
                nc.tensor.matmul(ps[:, 0:SP], wq_sb[:, k, :], xs[:, k, :],
                                 start=(k == 0), stop=(k == KT - 1))
            nc.vector.tensor_scalar_add(q2s[s][:], ps[:, 0:SP], bq_sb[:, 0:1])
            ps = pp.tile([P, SQ], F32, tag="ps_s", name="ps_k")
            for k in range(KT):
                nc.tensor.matmul(ps[:, 0:SP], wk_sb[:, k, :], xs[:, k, :],
                                 start=(k == 0), stop=(k == KT - 1))
            for h in range(HPC):
                hsl = slice(h * D, (h + 1) * D)
                nc.vector.tensor_scalar_add(k2zs[h][s][hsl, :], ps[hsl, 0:SP],
                                            bk_sb[hsl, 0:1])
            # v: project then transpose into vaug
            ps = pp.tile([P, SQ], F32, tag="ps_s", name="ps_v")
            for k in range(KT):
                nc.tensor.matmul(ps[:, 0:SP], wv_sb[:, k, :], xs[:, k, :],
                                 start=(k == 0), stop=(k == KT - 1))
            v2Ts = lp.tile([P, SP], F32, tag="v2Ts", name="v2Ts")
            nc.vector.tensor_scalar_add(v2Ts[:], ps[:, 0:SP], bv_sb[:, 0:1])
            for b in range(SP // P):
                t = s * (SP // P) + b
                ps_t = pp.tile([P, SQ], F32, tag="ps_s", name="ps_t")
                nc.tensor.transpose(ps_t[:, 0:P], v2Ts[:, b * P:(b + 1) * P],
                                    ident[:])
                for h in range(HPC):
                    nc.vector.tensor_copy(vaug[h][:, t, 0:D],
                                          ps_t[:, h * D:h * D + D])

        def emit_attn_tiles(sq, ps_o, mks):
            for mk in mks:
                use_pe = (mk % 2 == 0) or MASK_ALL_PE
                if mk % 2 == 0:
                    mt = wp.tile([P, SQ], F32R, tag="mta", name="mta")
                    nc.sync.dma_start(
                        mt[:], maskA[(mk // 2) * P:(mk // 2 + 1) * P,
                                     sq * SQ:(sq + 1) * SQ])
                else:
                    mt = wp.tile([P, SQ], F32R if MASK_ALL_PE else F32,
                                 tag="mtm", name="mtm")
                    nc.sync.dma_start(
                        mt[:], maskM[(mk // 2) * P:(mk // 2 + 1) * P,
                                     sq * SQ:(sq + 1) * SQ])
                for h in range(HPC):
                    ps_s = pp.tile([P, SQ], F32, tag="ps_s", name="ps_s")
                    for half in range(SQ // SP):
                        fsl = slice(half * SP, (half + 1) * SP)
                        nc.tensor.matmul(
                            ps_s[:, fsl],
                            k2zs[h][mk // 4][:, (mk % 4) * P:(mk % 4 + 1) * P],
                            q2s[sq * (SQ // SP) + half][:],
                            start=True, stop=not use_pe,
                        )
                        if use_pe:
                            nc.tensor.matmul(
                                ps_s[:, fsl], identR[:], mt[:, fsl],
                                start=False, stop=True,
                            )
                    p = wp.tile([P, SQ], F32R, tag="p", name="p")
                    nc.scalar.activation(p[:], ps_s[:], AF.Exp, scale=SCALE)
                    if not use_pe:
                        pm = wp.tile([P, SQ], F32R, tag="pm", name="pm")
                        nc.vector.tensor_tensor(pm[:], p[:], mt[:], ALU.mult)
                        p = pm
                    for half in range(SQ // SP):
                        fsl = slice(half * SP, (half + 1) * SP)
                        nc.tensor.matmul(
                            ps_o[h][:, fsl],
                            vaug[h][:, mk, :],
                            p[:, fsl],
                            start=(mk == 0), stop=(mk == NT - 1),
                        )

        def emit_epilogue(sq, ps_o):
            bcs = []
            for h in range(HPC):
                zrow = ep.tile([1, SQ], F32, tag="zrow", name=f"zrow{h}")
                nc.vector.tensor_copy(zrow[:], ps_o[h][D:D + 1, :])
                recip = ep.tile([1, SQ], F32, tag="recip", name=f"recip{h}")
                nc.vector.reciprocal_approx_fast(recip[:], zrow[:])
                bc = ep.tile([D, SQ], F32, tag=f"bc{h}", name=f"bc{h}")
                nc.gpsimd.partition_broadcast(bc[:], recip[:])
                bcs.append(bc)
            # normalize per node tile so out_proj starts early
            for b in range(SQ // P):
                t = sq * (SQ // P) + b
                for h in range(HPC):
                    nc.vector.tensor_tensor(
                        attn_t[t][h * D:(h + 1) * D, :],
                        ps_o[h][0:D, b * P:(b + 1) * P],
                        bcs[h][:, b * P:(b + 1) * P], ALU.mult)
                ps_y = pp.tile([P, SQ], F32, tag="ps_s", name="ps_y")
                for f in range(OUT_F // 512):
                    nc.tensor.matmul(ps_y[:, f * 512:(f + 1) * 512],
                                     attn_t[t][:, :],
                                     wo_sb[:, f * 512:(f + 1) * 512],
                                     start=True, stop=True)
                ys = wp.tile([P, OUT_F], F32, tag="ys", name="ys")
                if b % 2 == 0:
                    nc.vector.tensor_copy(ys[:], ps_y[:])
                else:
                    nc.scalar.copy(ys[:], ps_y[:])
                nc.sync.dma_start(ypart[t * P:(t + 1) * P, :], ys[:])

        # ---- interleaved emission: A strips feed B(sq=0) chunks ----
        emit_proj_strip(0)
        emit_proj_strip(1)
        ps_o0 = [pso.tile([D + 1, SQ], F32, tag=f"ps_o{h}", name=f"ps_o{h}")
                 for h in range(HPC)]
        emit_attn_tiles(0, ps_o0, range(0, 8))
        emit_proj_strip(2)
        emit_attn_tiles(0, ps_o0, range(8, 12))
        emit_proj_strip(3)
        emit_attn_tiles(0, ps_o0, range(12, 16))
        emit_proj_strip(4)
        emit_attn_tiles(0, ps_o0, range(16, 20))
        emit_proj_strip(5)
        emit_attn_tiles(0, ps_o0, range(20, 24))
        emit_epilogue(0, ps_o0)
        for sq in range(1, NSQ):
            ps_o = [pso.tile([D + 1, SQ], F32, tag=f"ps_o{h}", name=f"ps_o{h}")
                    for h in range(HPC)]
            emit_attn_tiles(sq, ps_o, range(NT))
            emit_epilogue(sq, ps_o)

    nc.compile()
    return nc


_PROGRAM = None
LAST_RESULTS = None


def _get_program():
    global _PROGRAM
    if _PROGRAM is None:
        _PROGRAM = build_program()
    return _PROGRAM


def _softplus(x):
    x = np.asarray(x, np.float32)
    return np.logaddexp(0.0, x).astype(np.float32)


def host_prep(inputs):
    x = np.asarray(inputs["x"], np.float32)
    edge_index = np.asarray(inputs["edge_index"])
    edge_type = np.asarray(inputs["edge_type"])
    etw = np.asarray(inputs["edge_type_weights"], np.float32)

    def f32(k):
        return np.asarray(inputs[k], np.float32)

    # compose the two linear layers: q2 = x @ (wiq@wq).T + (wiq@bq + biq)
    WQ = f32("wiq") @ f32("wq")
    bQ = f32("wiq") @ f32("bq") + f32("biq")
    WK = f32("wik") @ f32("wk")
    bK = f32("wik") @ f32("bk") + f32("bik")
    WV = f32("wiv") @ f32("wv")
    bV = f32("wiv") @ f32("bv") + f32("biv")
    wo = f32("wo")
    bo = f32("bo")

    # multiplicative mask, transposed: maskT[m, n] = exp(add_mask[n, m])
    w = _softplus(etw)
    NEG = np.float32(-8e30)
    M = np.full((N, N), NEG, dtype=np.float32)
    src, dst = edge_index[0], edge_index[1]
    wv8 = (w * np.float32(1.0 / SCALE)).astype(np.float32)
    M[src, dst] = wv8[edge_type - 1]           # last write wins, like jax .at[].set
    diag = np.diagonal(M).copy()
    didx = np.arange(N)
    M[didx, didx] = np.where(diag == NEG, wv8[3], diag)
    MT = np.ascontiguousarray(M.T)             # [key m, query n], additive * 8
    # even key tiles use the additive form on the PE, odd tiles the
    # multiplicative exp-form on the DVE
    MT4 = MT.reshape(NT, P, N)
    maskA = np.ascontiguousarray(MT4[0::2].reshape(N // 2, N))
    if MASK_ALL_PE:
        maskM = np.ascontiguousarray(MT4[1::2].reshape(N // 2, N))
    else:
        maskM = np.exp(MT4[1::2].reshape(N // 2, N).astype(np.float64)
                       * np.float64(SCALE)).astype(np.float32)

    xT = np.ascontiguousarray(x.T)

    in_maps = []
    for c in range(NCORES):
        rs = slice(c * CW, (c + 1) * CW)
        in_maps.append({
            "xT": xT,
            "maskA": maskA,
            "maskM": maskM,
            "wqT": np.ascontiguousarray(WQ[rs].T),
            "wkT": np.ascontiguousarray(WK[rs].T),
            "wvT": np.ascontiguousarray(WV[rs].T),
            "bq": np.ascontiguousarray(bQ[rs]),
            "bk": np.ascontiguousarray(bK[rs]),
            "bv": np.ascontiguousarray(bV[rs]),
            "woT": np.ascontiguousarray(wo[:, rs].T),
        })
    return in_maps, bo


def kernel(**inputs) -> np.ndarray:
    global LAST_RESULTS
    in_maps, bo = host_prep(inputs)
    nc = _get_program()
    trace = bool(os.environ.get("KERNEL_TRACE"))
    res = run_bass_kernel_spmd(nc, in_maps, list(range(NCORES)), trace=trace)
    LAST_RESULTS = res
    y = bo[None, :].astype(np.float32).repeat(N, axis=0)
    for c in range(NCORES):
        y += res.results[c]["ypart"]
    return y


# revision 27
# speedup vs baseline: 1.0530x; 1.0083x over previous
"""Sparse multi-head attention (nn_MultiHeadAttention_44332652429419) on 8 trn2 cores.

Strategy (tensor-parallel over H=16 heads, 2 heads per core):
  Host: compose the two stacked linear layers (q/k/v_proj followed by
        MultiheadAttention in_proj) into one weight per tensor; build the
        dense multiplicative mask exp(additive_mask) transposed; transpose x.
  Device (per core, SPMD with per-core weight slices):
    q2T/k2T/v2T = W_c @ x.T + b_c           [128, 3072] (2 heads x 64 dims)
    scoresT[mk,nq] = k2T_h.T-slice @ q2T_h  (K=64, two heads row-packed in PE)
    P = exp(scoresT * 1/8) * maskT          (ACT exp from PSUM, DVE multiply)
    outT_aug = [v_h | 1].T @ P              (rowsum via ones-augmented V)
    attnT = outT[:64] / outT[64]            (DVE recip + partition broadcast)
    ypart = attnT.T-slices @ woT_c          (partial out_proj, K=128)
  Host: y = sum_c ypart_c + bo
"""
import os
import sys

sys.path.insert(0, "/opt/trn_rl_repo")

import numpy as np
from contextlib import ExitStack

import concourse.bass as bass
import concourse.bacc as bacc
import concourse.mybir as mybir
import concourse.tile as tile
from concourse.bass_utils import run_bass_kernel_spmd
from concourse.masks import make_identity

F32 = mybir.dt.float32
F32R = mybir.dt.float32r
BF16 = mybir.dt.bfloat16
AF = mybir.ActivationFunctionType
ALU = mybir.AluOpType

N = 3072
IN_F = 1024
OUT_F = 1024
H = 16
D = 64
NCORES = 8
HPC = H // NCORES            # heads per core = 2
CW = HPC * D                 # per-core width = 128
P = 128
NT = N // P                  # 24 node tiles
KT = IN_F // P               # 8 contraction tiles
SQ = 1024                    # query strip width (phase B)
NSQ = N // SQ                # 3 strips
SP = 512                     # proj strip width (phase A)
NSP = N // SP                # 6 strips
SCALE = 1.0 / 8.0            # 1/sqrt(D)

MASK_DT = F32R               # additive mask, pre-scaled by 1/SCALE
MASK_ALL_PE = False          # apply mask via PE identity-add for all tiles


def build_program():
    nc = bacc.Bacc()
    xT = nc.declare_dram_parameter("xT", [IN_F, N], F32R, isOutput=False)
    # additive mask (pre-scaled by 1/SCALE) for even key tiles, multiplicative
    # exp-mask for odd key tiles — hybrid PE/DVE mask application
    maskA = nc.declare_dram_parameter("maskA", [N // 2, N], F32R, isOutput=False)
    maskM = nc.declare_dram_parameter("maskM", [N // 2, N],
                                     F32R if MASK_ALL_PE else F32, isOutput=False)
    wqT = nc.declare_dram_parameter("wqT", [IN_F, CW], F32R, isOutput=False)
    wkT = nc.declare_dram_parameter("wkT", [IN_F, CW], F32R, isOutput=False)
    wvT = nc.declare_dram_parameter("wvT", [IN_F, CW], F32R, isOutput=False)
    bq = nc.declare_dram_parameter("bq", [CW], F32, isOutput=False)
    bk = nc.declare_dram_parameter("bk", [CW], F32, isOutput=False)
    bv = nc.declare_dram_parameter("bv", [CW], F32, isOutput=False)
    woT = nc.declare_dram_parameter("woT", [CW, OUT_F], F32R, isOutput=False)
    ypart = nc.declare_dram_parameter("ypart", [N, OUT_F], F32, isOutput=True)

    with tile.TileContext(nc) as tc, ExitStack() as ctx:
        cst = ctx.enter_context(tc.tile_pool(name="cst", bufs=1))
        lp = ctx.enter_context(tc.tile_pool(name="lp", bufs=2))       # xs/v2Ts
        wp = ctx.enter_context(tc.tile_pool(name="wp", bufs=3))       # loop tiles
        ep = ctx.enter_context(tc.tile_pool(name="ep", bufs=1))       # epilogue
        pp = ctx.enter_context(tc.tile_pool(name="pp", bufs=2, space="PSUM"))
        pso = ctx.enter_context(tc.tile_pool(name="pso", bufs=1, space="PSUM"))

        ident = cst.tile([P, P], F32)
        make_identity(nc, ident)
        identR = cst.tile([P, P], F32R)
        nc.vector.tensor_copy(identR[:], ident[:])

        # per-512-strip persistent tensors (fine-grained deps let phase B
        # start while projections still run)
        q2s = [cst.tile([P, SP], F32R, tag=f"q2s{s}", name=f"q2s{s}")
               for s in range(NSP)]
        # k2z[h][s]: only rows h*D..h*D+63 live, rest zero — score matmuls
        # contract over full K=128 (keeps the PE HAM activity monitor warm)
        k2zs = [[cst.tile([P, SP], F32R, tag=f"k2z{h}_{s}", name=f"k2z{h}_{s}")
                 for s in range(NSP)] for h in range(HPC)]
        attn_t = [cst.tile([P, P], F32R, tag=f"attn{t}", name=f"attn{t}")
                  for t in range(NT)]
        vaug = [cst.tile([P, NT, D + 1], F32R, tag=f"vaug{h}", name=f"vaug{h}")
                for h in range(HPC)]
        ones_col = cst.tile([P, 1], F32)
        nc.vector.memset(ones_col[:], 1.0)
        zero_col = cst.tile([P, 1], F32)
        nc.vector.memset(zero_col[:], 0.0)
        for h in range(HPC):
            nc.vector.tensor_copy(vaug[h][:, :, D:D + 1],
                                  ones_col[:, 0:1, None].to_broadcast([P, NT, 1]))
            osl = slice((1 - h) * D, (2 - h) * D)   # the dead half of k2z[h]
            for s in range(NSP):
                nc.vector.tensor_copy(k2zs[h][s][osl, :],
                                      zero_col[osl, 0:1].to_broadcast([D, SP]))

        # weights
        wq_sb = cst.tile([P, KT, CW], F32R)
        nc.sync.dma_start(wq_sb[:], wqT.rearrange("(k p) m -> p k m", p=P))
        wk_sb = cst.tile([P, KT, CW], F32R)
        nc.sync.dma_start(wk_sb[:], wkT.rearrange("(k p) m -> p k m", p=P))
        wv_sb = cst.tile([P, KT, CW], F32R)
        nc.sync.dma_start(wv_sb[:], wvT.rearrange("(k p) m -> p k m", p=P))
        wo_sb = cst.tile([P, OUT_F], F32R)
        nc.sync.dma_start(wo_sb[:], woT[:])
        bq_sb = cst.tile([P, 1], F32)
        nc.sync.dma_start(bq_sb[:], bq[:, None])
        bk_sb = cst.tile([P, 1], F32)
        nc.sync.dma_start(bk_sb[:], bk[:, None])
        bv_sb = cst.tile([P, 1], F32)
        nc.sync.dma_start(bv_sb[:], bv[:, None])

        # ---- emission helpers ----
        def emit_proj_strip(s):
            xs = lp.tile([P, KT, SP], F32R, tag="xs", name="xs")
            nc.sync.dma_start(
                xs[:], xT.rearrange("(k p) n -> p k n", p=P)[:, :, s * SP:(s + 1) * SP])
            ps = pp.tile([P, SQ], F32, tag="ps_s", name="ps_q")
            for k in range(KT):
                nc.tensor.matmul(ps[:, 0:SP], wq_sb[:, k, :], xs[:, k, :],
                                 start=(k == 0), stop=(k == KT - 1))
            nc.vector.tensor_scalar_add(q2s[s][:], ps[:, 0:SP], bq_sb[:, 0:1])
            ps = pp.tile([P, SQ], F32, tag="ps_s", name="ps_k")
            for k in range(KT):
                nc.tensor.matmul(ps[:, 0:SP], wk_sb[:, k, :], xs[:, k, :],
                                 start=(k == 0), stop=(k == KT - 1))
            for h in range(HPC):
                hsl = slice(h * D, (h + 1) * D)
                nc.vector.tensor_scalar_add(k2zs[h][s][hsl, :], ps[hsl, 0:SP],
                                            bk_sb[hsl, 0:1])
            # v: project then transpose into vaug
            ps = pp.tile([P, SQ], F32, tag="ps_s", name="ps_v")
            for k in range(KT):
                nc.tensor.matmul(ps[:, 0:SP], wv_sb[:, k, :], xs[:, k, :],
                                 start=(k == 0), stop=(k == KT - 1))
            v2Ts = lp.tile([P, SP], F32, tag="v2Ts", name="v2Ts")
            nc.vector.tensor_scalar_add(v2Ts[:], ps[:, 0:SP], bv_sb[:, 0:1])
            for b in range(SP // P):
                t = s * (SP // P) + b
                ps_t = pp.tile([P, SQ], F32, tag="ps_s", name="ps_t")
                nc.tensor.transpose(ps_t[:, 0:P], v2Ts[:, b * P:(b + 1) * P],
                                    ident[:])
                for h in range(HPC):
                    nc.vector.tensor_copy(vaug[h][:, t, 0:D],
                                          ps_t[:, h * D:h * D + D])

        def emit_attn_tiles(sq, ps_o, mks):
            for mk in mks:
                use_pe = (mk % 2 == 0) or MASK_ALL_PE
                if mk % 2 == 0:
                    mt = wp.tile([P, SQ], F32R, tag="mta", name="mta")
                    nc.sync.dma_start(
                        mt[:], maskA[(mk // 2) * P:(mk // 2 + 1) * P,
                                     sq * SQ:(sq + 1) * SQ])
                else:
                    mt = wp.tile([P, SQ], F32R if MASK_ALL_PE else F32,
                                 tag="mtm", name="mtm")
                    nc.sync.dma_start(
                        mt[:], maskM[(mk // 2) * P:(mk // 2 + 1) * P,
                                     sq * SQ:(sq + 1) * SQ])
                for h in range(HPC):
                    ps_s = pp.tile([P, SQ], F32, tag="ps_s", name="ps_s")
                    for half in range(SQ // SP):
                        fsl = slice(half * SP, (half + 1) * SP)
                        nc.tensor.matmul(
                            ps_s[:, fsl],
                            k2zs[h][mk // 4][:, (mk % 4) * P:(mk % 4 + 1) * P],
                            q2s[sq * (SQ // SP) + half][:],
                            start=True, stop=not use_pe,
                        )
                        if use_pe:
                            nc.tensor.matmul(
                                ps_s[:, fsl], identR[:], mt[:, fsl],
                                start=False, stop=True,
                            )
                    p = wp.tile([P, SQ], F32R, tag="p", name="p")
                    nc.scalar.activation(p[:], ps_s[:], AF.Exp, scale=SCALE)
                    if not use_pe:
                        pm = wp.tile([P, SQ], F32R, tag="pm", name="pm")
                        nc.vector.tensor_tensor(pm[:], p[:], mt[:], ALU.mult)
                        p = pm
                    for half in range(SQ // SP):
                        fsl = slice(half * SP, (half + 1) * SP)
                        nc.tensor.matmul(
                            ps_o[h][:, fsl],
                            vaug[h][:, mk, :],
                            p[:, fsl],
                            start=(mk == 0), stop=(mk == NT - 1),
                        )

        def emit_epilogue(sq, ps_o):
            # stage PSUM accumulators to SBUF immediately so the next strip's
            # PV matmuls get the banks back as early as possible
            osb, bcs = [], []
            for h in range(HPC):
                ob = ep.tile([D + 1, SQ], F32, tag=f"osb{h}", name=f"osb{h}")
                nc.vector.tensor_copy(ob[:], ps_o[h][:])
                osb.append(ob)
            for h in range(HPC):
                zrow = ep.tile([1, SQ], F32, tag="zrow", name=f"zrow{h}")
                nc.vector.tensor_copy(zrow[:], osb[h][D:D + 1, :])
                recip = ep.tile([1, SQ], F32, tag="recip", name=f"recip{h}")
                nc.vector.reciprocal_approx_fast(recip[:], zrow[:])
                bc = ep.tile([D, SQ], F32, tag=f"bc{h}", name=f"bc{h}")
                nc.gpsimd.partition_broadcast(bc[:], recip[:])
                bcs.append(bc)
            # normalize per node tile so out_proj starts early
            for b in range(SQ // P):
                t = sq * (SQ // P) + b
                for h in range(HPC):
                    nc.vector.tensor_tensor(
                        attn_t[t][h * D:(h + 1) * D, :],
                        osb[h][0:D, b * P:(b + 1) * P],
                        bcs[h][:, b * P:(b + 1) * P], ALU.mult)
                ps_y = pp.tile([P, SQ], F32, tag="ps_s", name="ps_y")
                for f in range(OUT_F // 512):
                    nc.tensor.matmul(ps_y[:, f * 512:(f + 1) * 512],
                                     attn_t[t][:, :],
                                     wo_sb[:, f * 512:(f + 1) * 512],
                                     start=True, stop=True)
                ys = wp.tile([P, OUT_F], F32, tag="ys", name="ys")
                if b % 2 == 0:
                    nc.vector.tensor_copy(ys[:], ps_y[:])
                else:
                    nc.scalar.copy(ys[:], ps_y[:])
                nc.sync.dma_start(ypart[t * P:(t + 1) * P, :], ys[:])

        # ---- interleaved emission: A strips feed B(sq=0) chunks ----
        emit_proj_strip(0)
        emit_proj_strip(1)
        ps_o0 = [pso.tile([D + 1, SQ], F32, tag=f"ps_o{h}", name=f"ps_o{h}")
                 for h in range(HPC)]
        emit_attn_tiles(0, ps_o0, range(0, 8))
        emit_proj_strip(2)
        emit_attn_tiles(0, ps_o0, range(8, 12))
        emit_proj_strip(3)
        emit_attn_tiles(0, ps_o0, range(12, 16))
        emit_proj_strip(4)
        emit_attn_tiles(0, ps_o0, range(16, 20))
        emit_proj_strip(5)
        emit_attn_tiles(0, ps_o0, range(20, 24))
        ps_o_cur = ps_o0
        for sq in range(1, NSQ):
            ps_o_nxt = [pso.tile([D + 1, SQ], F32, tag=f"ps_o{h}",
                                 name=f"ps_o{h}_{sq}") for h in range(HPC)]
            emit_attn_tiles(sq, ps_o_nxt, range(0, 4))
            emit_epilogue(sq - 1, ps_o_cur)
            emit_attn_tiles(sq, ps_o_nxt, range(4, NT))
            ps_o_cur = ps_o_nxt
        emit_epilogue(NSQ - 1, ps_o_cur)

    nc.compile()
    return nc


_PROGRAM = None
LAST_RESULTS = None


def _get_program():
    global _PROGRAM
    if _PROGRAM is None:
        _PROGRAM = build_program()
    return _PROGRAM


def _softplus(x):
    x = np.asarray(x, np.float32)
    return np.logaddexp(0.0, x).astype(np.float32)


def host_prep(inputs):
    x = np.asarray(inputs["x"], np.float32)
    edge_index = np.asarray(inputs["edge_index"])
    edge_type = np.asarray(inputs["edge_type"])
    etw = np.asarray(inputs["edge_type_weights"], np.float32)

    def f32(k):
        return np.asarray(inputs[k], np.float32)

    # compose the two linear layers: q2 = x @ (wiq@wq).T + (wiq@bq + biq)
    WQ = f32("wiq") @ f32("wq")
    bQ = f32("wiq") @ f32("bq") + f32("biq")
    WK = f32("wik") @ f32("wk")
    bK = f32("wik") @ f32("bk") + f32("bik")
    WV = f32("wiv") @ f32("wv")
    bV = f32("wiv") @ f32("bv") + f32("biv")
    wo = f32("wo")
    bo = f32("bo")

    # multiplicative mask, transposed: maskT[m, n] = exp(add_mask[n, m])
    w = _softplus(etw)
    NEG = np.float32(-8e30)
    M = np.full((N, N), NEG, dtype=np.float32)
    src, dst = edge_index[0], edge_index[1]
    wv8 = (w * np.float32(1.0 / SCALE)).astype(np.float32)
    M[src, dst] = wv8[edge_type - 1]           # last write wins, like jax .at[].set
    diag = np.diagonal(M).copy()
    didx = np.arange(N)
    M[didx, didx] = np.where(diag == NEG, wv8[3], diag)
    MT = np.ascontiguousarray(M.T)             # [key m, query n], additive * 8
    # even key tiles use the additive form on the PE, odd tiles the
    # multiplicative exp-form on the DVE
    MT4 = MT.reshape(NT, P, N)
    maskA = np.ascontiguousarray(MT4[0::2].reshape(N // 2, N))
    if MASK_ALL_PE:
        maskM = np.ascontiguousarray(MT4[1::2].reshape(N // 2, N))
    else:
        maskM = np.exp(MT4[1::2].reshape(N // 2, N).astype(np.float64)
                       * np.float64(SCALE)).astype(np.float32)

    xT = np.ascontiguousarray(x.T)

    in_maps = []
    for c in range(NCORES):
        rs = slice(c * CW, (c + 1) * CW)
        in_maps.append({
            "xT": xT,
            "maskA": maskA,
            "maskM": maskM,
            "wqT": np.ascontiguousarray(WQ[rs].T),
            "wkT": np.ascontiguousarray(WK[rs].T),
            "wvT": np.ascontiguousarray(WV[rs].T),
            "bq": np.ascontiguousarray(bQ[rs]),
            "bk": np.ascontiguousarray(bK[rs]),
            "bv": np.ascontiguousarray(bV[rs]),
            "woT": np.ascontiguousarray(wo[:, rs].T),
        })
    return in_maps, bo


def kernel(**inputs) -> np.ndarray:
    global LAST_RESULTS
    in_maps, bo = host_prep(inputs)
    nc = _get_program()
    trace = bool(os.environ.get("KERNEL_TRACE"))
    res = run_bass_kernel_spmd(nc, in_maps, list(range(NCORES)), trace=trace)
    LAST_RESULTS = res
    y = bo[None, :].astype(np.float32).repeat(N, axis=0)
    for c in range(NCORES):
        y += res.results[c]["ypart"]
    return y


# revision 28
# speedup vs baseline: 1.0624x; 1.0089x over previous
"""Sparse multi-head attention (nn_MultiHeadAttention_44332652429419) on 8 trn2 cores.

Strategy (tensor-parallel over H=16 heads, 2 heads per core):
  Host: compose the two stacked linear layers (q/k/v_proj followed by
        MultiheadAttention in_proj) into one weight per tensor; build the
        dense multiplicative mask exp(additive_mask) transposed; transpose x.
  Device (per core, SPMD with per-core weight slices):
    q2T/k2T/v2T = W_c @ x.T + b_c           [128, 3072] (2 heads x 64 dims)
    scoresT[mk,nq] = k2T_h.T-slice @ q2T_h  (K=64, two heads row-packed in PE)
    P = exp(scoresT * 1/8) * maskT          (ACT exp from PSUM, DVE multiply)
    outT_aug = [v_h | 1].T @ P              (rowsum via ones-augmented V)
    attnT = outT[:64] / outT[64]            (DVE recip + partition broadcast)
    ypart = attnT.T-slices @ woT_c          (partial out_proj, K=128)
  Host: y = sum_c ypart_c + bo
"""
import os
import sys

sys.path.insert(0, "/opt/trn_rl_repo")

import numpy as np
from contextlib import ExitStack

import concourse.bass as bass
import concourse.bacc as bacc
import concourse.mybir as mybir
import concourse.tile as tile
from concourse.bass_utils import run_bass_kernel_spmd
from concourse.masks import make_identity

F32 = mybir.dt.float32
F32R = mybir.dt.float32r
BF16 = mybir.dt.bfloat16
F16 = mybir.dt.float16
AF = mybir.ActivationFunctionType
ALU = mybir.AluOpType

N = 3072
IN_F = 1024
OUT_F = 1024
H = 16
D = 64
NCORES = 8
HPC = H // NCORES            # heads per core = 2
CW = HPC * D                 # per-core width = 128
P = 128
NT = N // P                  # 24 node tiles
KT = IN_F // P               # 8 contraction tiles
SQ = 1024                    # query strip width (phase B)
NSQ = N // SQ                # 3 strips
SP = 512                     # proj strip width (phase A)
NSP = N // SP                # 6 strips
SCALE = 1.0 / 8.0            # 1/sqrt(D)

MASK_DT = F32R               # additive mask, pre-scaled by 1/SCALE
MASK_ALL_PE = False          # apply mask via PE identity-add for all tiles


def build_program():
    nc = bacc.Bacc()
    xT = nc.declare_dram_parameter("xT", [IN_F, N], F32R, isOutput=False)
    # additive mask (pre-scaled by 1/SCALE) for even key tiles, multiplicative
    # exp-mask for odd key tiles — hybrid PE/DVE mask application
    maskA = nc.declare_dram_parameter("maskA", [N // 2, N], F16, isOutput=False)
    maskM = nc.declare_dram_parameter("maskM", [N // 2, N], F16, isOutput=False)
    wqT = nc.declare_dram_parameter("wqT", [IN_F, CW], F32R, isOutput=False)
    wkT = nc.declare_dram_parameter("wkT", [IN_F, CW], F32R, isOutput=False)
    wvT = nc.declare_dram_parameter("wvT", [IN_F, CW], F32R, isOutput=False)
    bq = nc.declare_dram_parameter("bq", [CW], F32, isOutput=False)
    bk = nc.declare_dram_parameter("bk", [CW], F32, isOutput=False)
    bv = nc.declare_dram_parameter("bv", [CW], F32, isOutput=False)
    woT = nc.declare_dram_parameter("woT", [CW, OUT_F], F32R, isOutput=False)
    ypart = nc.declare_dram_parameter("ypart", [N, OUT_F], F32, isOutput=True)

    with tile.TileContext(nc) as tc, ExitStack() as ctx:
        cst = ctx.enter_context(tc.tile_pool(name="cst", bufs=1))
        lp = ctx.enter_context(tc.tile_pool(name="lp", bufs=2))       # xs/v2Ts
        wp = ctx.enter_context(tc.tile_pool(name="wp", bufs=3))       # loop tiles
        ep = ctx.enter_context(tc.tile_pool(name="ep", bufs=1))       # epilogue
        pp = ctx.enter_context(tc.tile_pool(name="pp", bufs=2, space="PSUM"))
        pso = ctx.enter_context(tc.tile_pool(name="pso", bufs=1, space="PSUM"))

        ident = cst.tile([P, P], F32)
        make_identity(nc, ident)
        identR = cst.tile([P, P], F32R)
        nc.vector.tensor_copy(identR[:], ident[:])
        identH = cst.tile([P, P], F16)
        nc.vector.tensor_copy(identH[:], ident[:])

        # per-512-strip persistent tensors (fine-grained deps let phase B
        # start while projections still run)
        q2s = [cst.tile([P, SP], F32R, tag=f"q2s{s}", name=f"q2s{s}")
               for s in range(NSP)]
        # k2z[h][s]: only rows h*D..h*D+63 live, rest zero — score matmuls
        # contract over full K=128 (keeps the PE HAM activity monitor warm)
        k2zs = [[cst.tile([P, SP], F32R, tag=f"k2z{h}_{s}", name=f"k2z{h}_{s}")
                 for s in range(NSP)] for h in range(HPC)]
        attn_t = [cst.tile([P, P], F32R, tag=f"attn{t}", name=f"attn{t}")
                  for t in range(NT)]
        vaug = [cst.tile([P, NT, D + 1], F32R, tag=f"vaug{h}", name=f"vaug{h}")
                for h in range(HPC)]
        ones_col = cst.tile([P, 1], F32)
        nc.vector.memset(ones_col[:], 1.0)
        zero_col = cst.tile([P, 1], F32)
        nc.vector.memset(zero_col[:], 0.0)
        for h in range(HPC):
            nc.vector.tensor_copy(vaug[h][:, :, D:D + 1],
                                  ones_col[:, 0:1, None].to_broadcast([P, NT, 1]))
            osl = slice((1 - h) * D, (2 - h) * D)   # the dead half of k2z[h]
            for s in range(NSP):
                nc.vector.tensor_copy(k2zs[h][s][osl, :],
                                      zero_col[osl, 0:1].to_broadcast([D, SP]))

        # weights
        wq_sb = cst.tile([P, KT, CW], F32R)
        nc.sync.dma_start(wq_sb[:], wqT.rearrange("(k p) m -> p k m", p=P))
        wk_sb = cst.tile([P, KT, CW], F32R)
        nc.sync.dma_start(wk_sb[:], wkT.rearrange("(k p) m -> p k m", p=P))
        wv_sb = cst.tile([P, KT, CW], F32R)
        nc.sync.dma_start(wv_sb[:], wvT.rearrange("(k p) m -> p k m", p=P))
        wo_sb = cst.tile([P, OUT_F], F32R)
        nc.sync.dma_start(wo_sb[:], woT[:])
        bq_sb = cst.tile([P, 1], F32)
        nc.sync.dma_start(bq_sb[:], bq[:, None])
        bk_sb = cst.tile([P, 1], F32)
        nc.sync.dma_start(bk_sb[:], bk[:, None])
        bv_sb = cst.tile([P, 1], F32)
        nc.sync.dma_start(bv_sb[:], bv[:, None])

        # ---- emission helpers ----
        def emit_proj_strip(s):
            xs = lp.tile([P, KT, SP], F32R, tag="xs", name="xs")
            nc.sync.dma_start(
                xs[:], xT.rearrange("(k p) n -> p k n", p=P)[:, :, s * SP:(s + 1) * SP])
            ps = pp.tile([P, SQ], F32, tag="ps_s", name="ps_q")
            for k in range(KT):
                nc.tensor.matmul(ps[:, 0:SP], wq_sb[:, k, :], xs[:, k, :],
                                 start=(k == 0), stop=(k == KT - 1))
            nc.vector.tensor_scalar_add(q2s[s][:], ps[:, 0:SP], bq_sb[:, 0:1])
            ps = pp.tile([P, SQ], F32, tag="ps_s", name="ps_k")
            for k in range(KT):
                nc.tensor.matmul(ps[:, 0:SP], wk_sb[:, k, :], xs[:, k, :],
                                 start=(k == 0), stop=(k == KT - 1))
            for h in range(HPC):
                hsl = slice(h * D, (h + 1) * D)
                nc.vector.tensor_scalar_add(k2zs[h][s][hsl, :], ps[hsl, 0:SP],
                                            bk_sb[hsl, 0:1])
            # v: project then transpose into vaug
            ps = pp.tile([P, SQ], F32, tag="ps_s", name="ps_v")
            for k in range(KT):
                nc.tensor.matmul(ps[:, 0:SP], wv_sb[:, k, :], xs[:, k, :],
                                 start=(k == 0), stop=(k == KT - 1))
            v2Ts = lp.tile([P, SP], F32, tag="v2Ts", name="v2Ts")
            nc.vector.tensor_scalar_add(v2Ts[:], ps[:, 0:SP], bv_sb[:, 0:1])
            for b in range(SP // P):
                t = s * (SP // P) + b
                ps_t = pp.tile([P, SQ], F32, tag="ps_s", name="ps_t")
                nc.tensor.transpose(ps_t[:, 0:P], v2Ts[:, b * P:(b + 1) * P],
                                    ident[:])
                for h in range(HPC):
                    nc.vector.tensor_copy(vaug[h][:, t, 0:D],
                                          ps_t[:, h * D:h * D + D])

        def emit_attn_tiles(sq, ps_o, mks):
            for mk in mks:
                use_pe = (mk % 2 == 0) or MASK_ALL_PE
                if mk % 2 == 0:
                    mt = wp.tile([P, SQ], F16, tag="mta", name="mta")
                    nc.sync.dma_start(
                        mt[:], maskA[(mk // 2) * P:(mk // 2 + 1) * P,
                                     sq * SQ:(sq + 1) * SQ])
                else:
                    mt = wp.tile([P, SQ], F16, tag="mtm", name="mtm")
                    nc.sync.dma_start(
                        mt[:], maskM[(mk // 2) * P:(mk // 2 + 1) * P,
                                     sq * SQ:(sq + 1) * SQ])
                for h in range(HPC):
                    ps_s = pp.tile([P, SQ], F32, tag="ps_s", name="ps_s")
                    for half in range(SQ // SP):
                        fsl = slice(half * SP, (half + 1) * SP)
                        nc.tensor.matmul(
                            ps_s[:, fsl],
                            k2zs[h][mk // 4][:, (mk % 4) * P:(mk % 4 + 1) * P],
                            q2s[sq * (SQ // SP) + half][:],
                            start=True, stop=not use_pe,
                        )
                        if use_pe:
                            nc.tensor.matmul(
                                ps_s[:, fsl], identH[:], mt[:, fsl],
                                start=False, stop=True,
                            )
                    p = wp.tile([P, SQ], F32R, tag="p", name="p")
                    nc.scalar.activation(p[:], ps_s[:], AF.Exp, scale=SCALE)
                    if not use_pe:
                        pm = wp.tile([P, SQ], F32R, tag="pm", name="pm")
                        nc.vector.tensor_tensor(pm[:], p[:], mt[:], ALU.mult)
                        p = pm
                    for half in range(SQ // SP):
                        fsl = slice(half * SP, (half + 1) * SP)
                        nc.tensor.matmul(
                            ps_o[h][:, fsl],
                            vaug[h][:, mk, :],
                            p[:, fsl],
                            start=(mk == 0), stop=(mk == NT - 1),
                        )

        def emit_epilogue(sq, ps_o):
            # stage PSUM accumulators to SBUF immediately so the next strip's
            # PV matmuls get the banks back as early as possible
            osb, bcs = [], []
            for h in range(HPC):
                ob = ep.tile([D + 1, SQ], F32, tag=f"osb{h}", name=f"osb{h}")
                nc.vector.tensor_copy(ob[:], ps_o[h][:])
                osb.append(ob)
            for h in range(HPC):
                zrow = ep.tile([1, SQ], F32, tag="zrow", name=f"zrow{h}")
                nc.vector.tensor_copy(zrow[:], osb[h][D:D + 1, :])
                recip = ep.tile([1, SQ], F32, tag="recip", name=f"recip{h}")
                nc.vector.reciprocal_approx_fast(recip[:], zrow[:])
                bc = ep.tile([D, SQ], F32, tag=f"bc{h}", name=f"bc{h}")
                nc.gpsimd.partition_broadcast(bc[:], recip[:])
                bcs.append(bc)
            # normalize per node tile so out_proj starts early
            for b in range(SQ // P):
                t = sq * (SQ // P) + b
                for h in range(HPC):
                    nc.vector.tensor_tensor(
                        attn_t[t][h * D:(h + 1) * D, :],
                        osb[h][0:D, b * P:(b + 1) * P],
                        bcs[h][:, b * P:(b + 1) * P], ALU.mult)
                ps_y = pp.tile([P, SQ], F32, tag="ps_s", name="ps_y")
                for f in range(OUT_F // 512):
                    nc.tensor.matmul(ps_y[:, f * 512:(f + 1) * 512],
                                     attn_t[t][:, :],
                                     wo_sb[:, f * 512:(f + 1) * 512],
                                     start=True, stop=True)
                ys = wp.tile([P, OUT_F], F32, tag="ys", name="ys")
                if b % 2 == 0:
                    nc.vector.tensor_copy(ys[:], ps_y[:])
                else:
                    nc.scalar.copy(ys[:], ps_y[:])
                nc.sync.dma_start(ypart[t * P:(t + 1) * P, :], ys[:])

        # ---- interleaved emission: A strips feed B(sq=0) chunks ----
        emit_proj_strip(0)
        emit_proj_strip(1)
        ps_o0 = [pso.tile([D + 1, SQ], F32, tag=f"ps_o{h}", name=f"ps_o{h}")
                 for h in range(HPC)]
        emit_attn_tiles(0, ps_o0, range(0, 8))
        emit_proj_strip(2)
        emit_attn_tiles(0, ps_o0, range(8, 12))
        emit_proj_strip(3)
        emit_attn_tiles(0, ps_o0, range(12, 16))
        emit_proj_strip(4)
        emit_attn_tiles(0, ps_o0, range(16, 20))
        emit_proj_strip(5)
        emit_attn_tiles(0, ps_o0, range(20, 24))
        ps_o_cur = ps_o0
        for sq in range(1, NSQ):
            ps_o_nxt = [pso.tile([D + 1, SQ], F32, tag=f"ps_o{h}",
                                 name=f"ps_o{h}_{sq}") for h in range(HPC)]
            emit_attn_tiles(sq, ps_o_nxt, range(0, 4))
            emit_epilogue(sq - 1, ps_o_cur)
            emit_attn_tiles(sq, ps_o_nxt, range(4, NT))
            ps_o_cur = ps_o_nxt
        emit_epilogue(NSQ - 1, ps_o_cur)

    nc.compile()
    return nc


_PROGRAM = None
LAST_RESULTS = None


def _get_program():
    global _PROGRAM
    if _PROGRAM is None:
        _PROGRAM = build_program()
    return _PROGRAM


def _softplus(x):
    x = np.asarray(x, np.float32)
    return np.logaddexp(0.0, x).astype(np.float32)


def host_prep(inputs):
    x = np.asarray(inputs["x"], np.float32)
    edge_index = np.asarray(inputs["edge_index"])
    edge_type = np.asarray(inputs["edge_type"])
    etw = np.asarray(inputs["edge_type_weights"], np.float32)

    def f32(k):
        return np.asarray(inputs[k], np.float32)

    # compose the two linear layers: q2 = x @ (wiq@wq).T + (wiq@bq + biq)
    WQ = f32("wiq") @ f32("wq")
    bQ = f32("wiq") @ f32("bq") + f32("biq")
    WK = f32("wik") @ f32("wk")
    bK = f32("wik") @ f32("bk") + f32("bik")
    WV = f32("wiv") @ f32("wv")
    bV = f32("wiv") @ f32("bv") + f32("biv")
    wo = f32("wo")
    bo = f32("bo")

    # multiplicative mask, transposed: maskT[m, n] = exp(add_mask[n, m])
    w = _softplus(etw)
    NEG = np.float32(-60000.0)
    M = np.full((N, N), NEG, dtype=np.float32)
    src, dst = edge_index[0], edge_index[1]
    wv8 = (w * np.float32(1.0 / SCALE)).astype(np.float32)
    M[src, dst] = wv8[edge_type - 1]           # last write wins, like jax .at[].set
    diag = np.diagonal(M).copy()
    didx = np.arange(N)
    M[didx, didx] = np.where(diag == NEG, wv8[3], diag)
    MT = np.ascontiguousarray(M.T)             # [key m, query n], additive * 8
    # even key tiles use the additive form on the PE, odd tiles the
    # multiplicative exp-form on the DVE
    MT4 = MT.reshape(NT, P, N)
    maskA = MT4[0::2].reshape(N // 2, N).astype(np.float16)
    if MASK_ALL_PE:
        maskM = MT4[1::2].reshape(N // 2, N).astype(np.float16)
    else:
        maskM = np.exp(MT4[1::2].reshape(N // 2, N).astype(np.float64)
                       * np.float64(SCALE)).astype(np.float16)

    xT = np.ascontiguousarray(x.T)

    in_maps = []
    for c in range(NCORES):
        rs = slice(c * CW, (c + 1) * CW)
        in_maps.append({
            "xT": xT,
            "maskA": maskA,
            "maskM": maskM,
            "wqT": np.ascontiguousarray(WQ[rs].T),
            "wkT": np.ascontiguousarray(WK[rs].T),
            "wvT": np.ascontiguousarray(WV[rs].T),
            "bq": np.ascontiguousarray(bQ[rs]),
            "bk": np.ascontiguousarray(bK[rs]),
            "bv": np.ascontiguousarray(bV[rs]),
            "woT": np.ascontiguousarray(wo[:, rs].T),
        })
    return in_maps, bo


def kernel(**inputs) -> np.ndarray:
    global LAST_RESULTS
    in_maps, bo = host_prep(inputs)
    nc = _get_program()
    trace = bool(os.environ.get("KERNEL_TRACE"))
    res = run_bass_kernel_spmd(nc, in_maps, list(range(NCORES)), trace=trace)
    LAST_RESULTS = res
    y = bo[None, :].astype(np.float32).repeat(N, axis=0)
    for c in range(NCORES):
        y += res.results[c]["ypart"]
    return y


# revision 29
# speedup vs baseline: 1.0645x; 1.0020x over previous
"""Sparse multi-head attention (nn_MultiHeadAttention_44332652429419) on 8 trn2 cores.

Strategy (tensor-parallel over H=16 heads, 2 heads per core):
  Host: compose the two stacked linear layers (q/k/v_proj followed by
        MultiheadAttention in_proj) into one weight per tensor; build the
        dense multiplicative mask exp(additive_mask) transposed; transpose x.
  Device (per core, SPMD with per-core weight slices):
    q2T/k2T/v2T = W_c @ x.T + b_c           [128, 3072] (2 heads x 64 dims)
    scoresT[mk,nq] = k2T_h.T-slice @ q2T_h  (K=64, two heads row-packed in PE)
    P = exp(scoresT * 1/8) * maskT          (ACT exp from PSUM, DVE multiply)
    outT_aug = [v_h | 1].T @ P              (rowsum via ones-augmented V)
    attnT = outT[:64] / outT[64]            (DVE recip + partition broadcast)
    ypart = attnT.T-slices @ woT_c          (partial out_proj, K=128)
  Host: y = sum_c ypart_c + bo
"""
import os
import sys

sys.path.insert(0, "/opt/trn_rl_repo")

import numpy as np
from contextlib import ExitStack

import concourse.bass as bass
import concourse.bacc as bacc
import concourse.mybir as mybir
import concourse.tile as tile
from concourse.bass_utils import run_bass_kernel_spmd
from concourse.masks import make_identity

F32 = mybir.dt.float32
F32R = mybir.dt.float32r
BF16 = mybir.dt.bfloat16
F16 = mybir.dt.float16
AF = mybir.ActivationFunctionType
ALU = mybir.AluOpType

N = 3072
IN_F = 1024
OUT_F = 1024
H = 16
D = 64
NCORES = 8
HPC = H // NCORES            # heads per core = 2
CW = HPC * D                 # per-core width = 128
P = 128
NT = N // P                  # 24 node tiles
KT = IN_F // P               # 8 contraction tiles
SQ = 1024                    # query strip width (phase B)
NSQ = N // SQ                # 3 strips
SP = 512                     # proj strip width (phase A)
NSP = N // SP                # 6 strips
SCALE = 1.0 / 8.0            # 1/sqrt(D)

MASK_DT = F32R               # additive mask, pre-scaled by 1/SCALE
MASK_ALL_PE = False          # apply mask via PE identity-add for all tiles


def build_program():
    nc = bacc.Bacc()
    xT = nc.declare_dram_parameter("xT", [IN_F, N], F32R, isOutput=False)
    # additive mask (pre-scaled by 1/SCALE) for even key tiles, multiplicative
    # exp-mask for odd key tiles — hybrid PE/DVE mask application
    maskA = nc.declare_dram_parameter("maskA", [N // 2, N], F16, isOutput=False)
    maskM = nc.declare_dram_parameter("maskM", [N // 2, N], F16, isOutput=False)
    wqT = nc.declare_dram_parameter("wqT", [IN_F, CW], F32R, isOutput=False)
    wkT = nc.declare_dram_parameter("wkT", [IN_F, CW], F32R, isOutput=False)
    wvT = nc.declare_dram_parameter("wvT", [IN_F, CW], F32R, isOutput=False)
    bq = nc.declare_dram_parameter("bq", [CW], F32, isOutput=False)
    bk = nc.declare_dram_parameter("bk", [CW], F32, isOutput=False)
    bv = nc.declare_dram_parameter("bv", [CW], F32, isOutput=False)
    woT = nc.declare_dram_parameter("woT", [CW, OUT_F], F32R, isOutput=False)
    ypart = nc.declare_dram_parameter("ypart", [N, OUT_F], F32, isOutput=True)

    with tile.TileContext(nc) as tc, ExitStack() as ctx:
        cst = ctx.enter_context(tc.tile_pool(name="cst", bufs=1))
        lp = ctx.enter_context(tc.tile_pool(name="lp", bufs=2))       # xs/v2Ts
        wp = ctx.enter_context(tc.tile_pool(name="wp", bufs=4))       # loop tiles
        ep = ctx.enter_context(tc.tile_pool(name="ep", bufs=1))       # epilogue
        pp = ctx.enter_context(tc.tile_pool(name="pp", bufs=2, space="PSUM"))
        pso = ctx.enter_context(tc.tile_pool(name="pso", bufs=1, space="PSUM"))

        ident = cst.tile([P, P], F32)
        make_identity(nc, ident)
        identR = cst.tile([P, P], F32R)
        nc.vector.tensor_copy(identR[:], ident[:])
        identH = cst.tile([P, P], F16)
        nc.vector.tensor_copy(identH[:], ident[:])

        # per-512-strip persistent tensors (fine-grained deps let phase B
        # start while projections still run)
        q2s = [cst.tile([P, SP], F32R, tag=f"q2s{s}", name=f"q2s{s}")
               for s in range(NSP)]
        # k2z[h][s]: only rows h*D..h*D+63 live, rest zero — score matmuls
        # contract over full K=128 (keeps the PE HAM activity monitor warm)
        k2zs = [[cst.tile([P, SP], F32R, tag=f"k2z{h}_{s}", name=f"k2z{h}_{s}")
                 for s in range(NSP)] for h in range(HPC)]
        attn_t = [cst.tile([P, P], F32R, tag=f"attn{t}", name=f"attn{t}")
                  for t in range(NT)]
        vaug = [cst.tile([P, NT, D + 1], F32R, tag=f"vaug{h}", name=f"vaug{h}")
                for h in range(HPC)]
        ones_col = cst.tile([P, 1], F32)
        nc.vector.memset(ones_col[:], 1.0)
        zero_col = cst.tile([P, 1], F32)
        nc.vector.memset(zero_col[:], 0.0)
        for h in range(HPC):
            nc.vector.tensor_copy(vaug[h][:, :, D:D + 1],
                                  ones_col[:, 0:1, None].to_broadcast([P, NT, 1]))
            osl = slice((1 - h) * D, (2 - h) * D)   # the dead half of k2z[h]
            for s in range(NSP):
                nc.vector.tensor_copy(k2zs[h][s][osl, :],
                                      zero_col[osl, 0:1].to_broadcast([D, SP]))

        # weights
        wq_sb = cst.tile([P, KT, CW], F32R)
        nc.sync.dma_start(wq_sb[:], wqT.rearrange("(k p) m -> p k m", p=P))
        wk_sb = cst.tile([P, KT, CW], F32R)
        nc.sync.dma_start(wk_sb[:], wkT.rearrange("(k p) m -> p k m", p=P))
        wv_sb = cst.tile([P, KT, CW], F32R)
        nc.sync.dma_start(wv_sb[:], wvT.rearrange("(k p) m -> p k m", p=P))
        wo_sb = cst.tile([P, OUT_F], F32R)
        nc.sync.dma_start(wo_sb[:], woT[:])
        bq_sb = cst.tile([P, 1], F32)
        nc.sync.dma_start(bq_sb[:], bq[:, None])
        bk_sb = cst.tile([P, 1], F32)
        nc.sync.dma_start(bk_sb[:], bk[:, None])
        bv_sb = cst.tile([P, 1], F32)
        nc.sync.dma_start(bv_sb[:], bv[:, None])

        # ---- emission helpers ----
        def emit_proj_strip(s):
            xs = lp.tile([P, KT, SP], F32R, tag="xs", name="xs")
            nc.sync.dma_start(
                xs[:], xT.rearrange("(k p) n -> p k n", p=P)[:, :, s * SP:(s + 1) * SP])
            ps = pp.tile([P, SQ], F32, tag="ps_s", name="ps_q")
            for k in range(KT):
                nc.tensor.matmul(ps[:, 0:SP], wq_sb[:, k, :], xs[:, k, :],
                                 start=(k == 0), stop=(k == KT - 1))
            nc.vector.tensor_scalar_add(q2s[s][:], ps[:, 0:SP], bq_sb[:, 0:1])
            ps = pp.tile([P, SQ], F32, tag="ps_s", name="ps_k")
            for k in range(KT):
                nc.tensor.matmul(ps[:, 0:SP], wk_sb[:, k, :], xs[:, k, :],
                                 start=(k == 0), stop=(k == KT - 1))
            for h in range(HPC):
                hsl = slice(h * D, (h + 1) * D)
                nc.vector.tensor_scalar_add(k2zs[h][s][hsl, :], ps[hsl, 0:SP],
                                            bk_sb[hsl, 0:1])
            # v: project then transpose into vaug
            ps = pp.tile([P, SQ], F32, tag="ps_s", name="ps_v")
            for k in range(KT):
                nc.tensor.matmul(ps[:, 0:SP], wv_sb[:, k, :], xs[:, k, :],
                                 start=(k == 0), stop=(k == KT - 1))
            v2Ts = lp.tile([P, SP], F32, tag="v2Ts", name="v2Ts")
            nc.vector.tensor_scalar_add(v2Ts[:], ps[:, 0:SP], bv_sb[:, 0:1])
            for b in range(SP // P):
                t = s * (SP // P) + b
                ps_t = pp.tile([P, SQ], F32, tag="ps_s", name="ps_t")
                nc.tensor.transpose(ps_t[:, 0:P], v2Ts[:, b * P:(b + 1) * P],
                                    ident[:])
                for h in range(HPC):
                    nc.vector.tensor_copy(vaug[h][:, t, 0:D],
                                          ps_t[:, h * D:h * D + D])

        def emit_attn_tiles(sq, ps_o, mks):
            for mk in mks:
                use_pe = (mk % 2 == 0) or MASK_ALL_PE
                if mk % 2 == 0:
                    mt = wp.tile([P, SQ], F16, tag="mta", name="mta")
                    nc.sync.dma_start(
                        mt[:], maskA[(mk // 2) * P:(mk // 2 + 1) * P,
                                     sq * SQ:(sq + 1) * SQ])
                else:
                    mt = wp.tile([P, SQ], F16, tag="mtm", name="mtm")
                    nc.sync.dma_start(
                        mt[:], maskM[(mk // 2) * P:(mk // 2 + 1) * P,
                                     sq * SQ:(sq + 1) * SQ])
                for h in range(HPC):
                    ps_s = pp.tile([P, SQ], F32, tag="ps_s", name="ps_s")
                    for half in range(SQ // SP):
                        fsl = slice(half * SP, (half + 1) * SP)
                        nc.tensor.matmul(
                            ps_s[:, fsl],
                            k2zs[h][mk // 4][:, (mk % 4) * P:(mk % 4 + 1) * P],
                            q2s[sq * (SQ // SP) + half][:],
                            start=True, stop=not use_pe,
                        )
                        if use_pe:
                            nc.tensor.matmul(
                                ps_s[:, fsl], identH[:], mt[:, fsl],
                                start=False, stop=True,
                            )
                    p = wp.tile([P, SQ], F32R, tag="p", name="p")
                    nc.scalar.activation(p[:], ps_s[:], AF.Exp, scale=SCALE)
                    if not use_pe:
                        pm = wp.tile([P, SQ], F32R, tag="pm", name="pm")
                        nc.vector.tensor_tensor(pm[:], p[:], mt[:], ALU.mult)
                        p = pm
                    for half in range(SQ // SP):
                        fsl = slice(half * SP, (half + 1) * SP)
                        nc.tensor.matmul(
                            ps_o[h][:, fsl],
                            vaug[h][:, mk, :],
                            p[:, fsl],
                            start=(mk == 0), stop=(mk == NT - 1),
                        )

        def emit_epilogue(sq, ps_o):
            # stage PSUM accumulators to SBUF immediately so the next strip's
            # PV matmuls get the banks back as early as possible
            osb, bcs = [], []
            for h in range(HPC):
                ob = ep.tile([D + 1, SQ], F32, tag=f"osb{h}", name=f"osb{h}")
                nc.vector.tensor_copy(ob[:], ps_o[h][:])
                osb.append(ob)
            for h in range(HPC):
                zrow = ep.tile([1, SQ], F32, tag="zrow", name=f"zrow{h}")
                nc.vector.tensor_copy(zrow[:], osb[h][D:D + 1, :])
                recip = ep.tile([1, SQ], F32, tag="recip", name=f"recip{h}")
                nc.vector.reciprocal_approx_fast(recip[:], zrow[:])
                bc = ep.tile([D, SQ], F32, tag=f"bc{h}", name=f"bc{h}")
                nc.gpsimd.partition_broadcast(bc[:], recip[:])
                bcs.append(bc)
            # normalize per node tile so out_proj starts early
            for b in range(SQ // P):
                t = sq * (SQ // P) + b
                for h in range(HPC):
                    nc.vector.tensor_tensor(
                        attn_t[t][h * D:(h + 1) * D, :],
                        osb[h][0:D, b * P:(b + 1) * P],
                        bcs[h][:, b * P:(b + 1) * P], ALU.mult)
                ps_y = pp.tile([P, SQ], F32, tag="ps_s", name="ps_y")
                for f in range(OUT_F // 512):
                    nc.tensor.matmul(ps_y[:, f * 512:(f + 1) * 512],
                                     attn_t[t][:, :],
                                     wo_sb[:, f * 512:(f + 1) * 512],
                                     start=True, stop=True)
                ys = wp.tile([P, OUT_F], F32, tag="ys", name="ys")
                if b % 2 == 0:
                    nc.vector.tensor_copy(ys[:], ps_y[:])
                else:
                    nc.scalar.copy(ys[:], ps_y[:])
                nc.sync.dma_start(ypart[t * P:(t + 1) * P, :], ys[:])

        # ---- interleaved emission: A strips feed B(sq=0) chunks ----
        emit_proj_strip(0)
        emit_proj_strip(1)
        ps_o0 = [pso.tile([D + 1, SQ], F32, tag=f"ps_o{h}", name=f"ps_o{h}")
                 for h in range(HPC)]
        emit_attn_tiles(0, ps_o0, range(0, 8))
        emit_proj_strip(2)
        emit_attn_tiles(0, ps_o0, range(8, 12))
        emit_proj_strip(3)
        emit_attn_tiles(0, ps_o0, range(12, 16))
        emit_proj_strip(4)
        emit_attn_tiles(0, ps_o0, range(16, 20))
        emit_proj_strip(5)
        emit_attn_tiles(0, ps_o0, range(20, 24))
        ps_o_cur = ps_o0
        for sq in range(1, NSQ):
            ps_o_nxt = [pso.tile([D + 1, SQ], F32, tag=f"ps_o{h}",
                                 name=f"ps_o{h}_{sq}") for h in range(HPC)]
            emit_attn_tiles(sq, ps_o_nxt, range(0, 4))
            emit_epilogue(sq - 1, ps_o_cur)
            emit_attn_tiles(sq, ps_o_nxt, range(4, NT))
            ps_o_cur = ps_o_nxt
        emit_epilogue(NSQ - 1, ps_o_cur)

    nc.compile()
    return nc


_PROGRAM = None
LAST_RESULTS = None


def _get_program():
    global _PROGRAM
    if _PROGRAM is None:
        _PROGRAM = build_program()
    return _PROGRAM


def _softplus(x):
    x = np.asarray(x, np.float32)
    return np.logaddexp(0.0, x).astype(np.float32)


def host_prep(inputs):
    x = np.asarray(inputs["x"], np.float32)
    edge_index = np.asarray(inputs["edge_index"])
    edge_type = np.asarray(inputs["edge_type"])
    etw = np.asarray(inputs["edge_type_weights"], np.float32)

    def f32(k):
        return np.asarray(inputs[k], np.float32)

    # compose the two linear layers: q2 = x @ (wiq@wq).T + (wiq@bq + biq)
    WQ = f32("wiq") @ f32("wq")
    bQ = f32("wiq") @ f32("bq") + f32("biq")
    WK = f32("wik") @ f32("wk")
    bK = f32("wik") @ f32("bk") + f32("bik")
    WV = f32("wiv") @ f32("wv")
    bV = f32("wiv") @ f32("bv") + f32("biv")
    wo = f32("wo")
    bo = f32("bo")

    # multiplicative mask, transposed: maskT[m, n] = exp(add_mask[n, m])
    w = _softplus(etw)
    NEG = np.float32(-60000.0)
    M = np.full((N, N), NEG, dtype=np.float32)
    src, dst = edge_index[0], edge_index[1]
    wv8 = (w * np.float32(1.0 / SCALE)).astype(np.float32)
    M[src, dst] = wv8[edge_type - 1]           # last write wins, like jax .at[].set
    diag = np.diagonal(M).copy()
    didx = np.arange(N)
    M[didx, didx] = np.where(diag == NEG, wv8[3], diag)
    MT = np.ascontiguousarray(M.T)             # [key m, query n], additive * 8
    # even key tiles use the additive form on the PE, odd tiles the
    # multiplicative exp-form on the DVE
    MT4 = MT.reshape(NT, P, N)
    maskA = MT4[0::2].reshape(N // 2, N).astype(np.float16)
    if MASK_ALL_PE:
        maskM = MT4[1::2].reshape(N // 2, N).astype(np.float16)
    else:
        maskM = np.exp(MT4[1::2].reshape(N // 2, N).astype(np.float64)
                       * np.float64(SCALE)).astype(np.float16)

    xT = np.ascontiguousarray(x.T)

    in_maps = []
    for c in range(NCORES):
        rs = slice(c * CW, (c + 1) * CW)
        in_maps.append({
            "xT": xT,
            "maskA": maskA,
            "maskM": maskM,
            "wqT": np.ascontiguousarray(WQ[rs].T),
            "wkT": np.ascontiguousarray(WK[rs].T),
            "wvT": np.ascontiguousarray(WV[rs].T),
            "bq": np.ascontiguousarray(bQ[rs]),
            "bk": np.ascontiguousarray(bK[rs]),
            "bv": np.ascontiguousarray(bV[rs]),
            "woT": np.ascontiguousarray(wo[:, rs].T),
        })
    return in_maps, bo


def kernel(**inputs) -> np.ndarray:
    global LAST_RESULTS
    in_maps, bo = host_prep(inputs)
    nc = _get_program()
    trace = bool(os.environ.get("KERNEL_TRACE"))
    res = run_bass_kernel_spmd(nc, in_maps, list(range(NCORES)), trace=trace)
    LAST_RESULTS = res
    y = bo[None, :].astype(np.float32).repeat(N, axis=0)
    for c in range(NCORES):
        y += res.results[c]["ypart"]
    return y


# revision 30
# speedup vs baseline: 1.1322x; 1.0636x over previous
"""Sparse multi-head attention (nn_MultiHeadAttention_44332652429419) on 8 trn2 cores.

Strategy (tensor-parallel over H=16 heads, 2 heads per core):
  Host: compose the two stacked linear layers (q/k/v_proj followed by
        MultiheadAttention in_proj) into one weight per tensor; build the
        dense multiplicative mask exp(additive_mask) transposed; transpose x.
  Device (per core, SPMD with per-core weight slices):
    q2T/k2T/v2T = W_c @ x.T + b_c           [128, 3072] (2 heads x 64 dims)
    scoresT[mk,nq] = k2T_h.T-slice @ q2T_h  (K=64, two heads row-packed in PE)
    P = exp(scoresT * 1/8) * maskT          (ACT exp from PSUM, DVE multiply)
    outT_aug = [v_h | 1].T @ P              (rowsum via ones-augmented V)
    attnT = outT[:64] / outT[64]            (DVE recip + partition broadcast)
    ypart = attnT.T-slices @ woT_c          (partial out_proj, K=128)
  Host: y = sum_c ypart_c + bo
"""
import os
import sys

sys.path.insert(0, "/opt/trn_rl_repo")

import numpy as np
from contextlib import ExitStack

import concourse.bass as bass
import concourse.bacc as bacc
import concourse.mybir as mybir
import concourse.tile as tile
from concourse.bass_utils import run_bass_kernel_spmd
from concourse.masks import make_identity

F32 = mybir.dt.float32
F32R = mybir.dt.float32r
BF16 = mybir.dt.bfloat16
F16 = mybir.dt.float16
AF = mybir.ActivationFunctionType
ALU = mybir.AluOpType

N = 3072
IN_F = 1024
OUT_F = 1024
H = 16
D = 64
NCORES = 8
HPC = H // NCORES            # heads per core = 2
CW = HPC * D                 # per-core width = 128
P = 128
NT = N // P                  # 24 node tiles
KT = IN_F // P               # 8 contraction tiles
SQ = 1024                    # query strip width (phase B)
NSQ = N // SQ                # 3 strips
SP = 512                     # proj strip width (phase A)
NSP = N // SP                # 6 strips
SCALE = 1.0 / 8.0            # 1/sqrt(D)

MASK_DT = F32R               # additive mask, pre-scaled by 1/SCALE
MASK_ALL_PE = False          # apply mask via PE identity-add for all tiles


def build_program():
    nc = bacc.Bacc()
    xT = nc.declare_dram_parameter("xT", [IN_F, N], F32R, isOutput=False)
    # additive mask (pre-scaled by 1/SCALE) for even key tiles, multiplicative
    # exp-mask for odd key tiles — hybrid PE/DVE mask application
    maskA = nc.declare_dram_parameter("maskA", [N // 2, N], F16, isOutput=False)
    maskM = nc.declare_dram_parameter("maskM", [N // 2, N], F16, isOutput=False)
    wqT = nc.declare_dram_parameter("wqT", [IN_F, CW], F32R, isOutput=False)
    wkT = nc.declare_dram_parameter("wkT", [IN_F, CW], F32R, isOutput=False)
    wvT = nc.declare_dram_parameter("wvT", [IN_F, CW], F32R, isOutput=False)
    bq = nc.declare_dram_parameter("bq", [CW], F32, isOutput=False)
    bk = nc.declare_dram_parameter("bk", [CW], F32, isOutput=False)
    bv = nc.declare_dram_parameter("bv", [CW], F32, isOutput=False)
    woT = nc.declare_dram_parameter("woT", [CW, OUT_F], F32R, isOutput=False)
    ypart = nc.declare_dram_parameter("ypart", [N, OUT_F], F32, isOutput=True)

    with tile.TileContext(nc) as tc, ExitStack() as ctx:
        cst = ctx.enter_context(tc.tile_pool(name="cst", bufs=1))
        lp = ctx.enter_context(tc.tile_pool(name="lp", bufs=2))       # xs/v2Ts
        wp = ctx.enter_context(tc.tile_pool(name="wp", bufs=4))       # loop tiles
        ep = ctx.enter_context(tc.tile_pool(name="ep", bufs=1))       # epilogue
        pp = ctx.enter_context(tc.tile_pool(name="pp", bufs=2, space="PSUM"))
        pso = ctx.enter_context(tc.tile_pool(name="pso", bufs=1, space="PSUM"))

        ident = cst.tile([P, P], F32)
        make_identity(nc, ident)
        identR = cst.tile([P, P], F32R)
        nc.vector.tensor_copy(identR[:], ident[:])
        identH = cst.tile([P, P], F16)
        nc.vector.tensor_copy(identH[:], ident[:])

        # per-512-strip persistent tensors (fine-grained deps let phase B
        # start while projections still run)
        q2s = [cst.tile([P, SP], F32R, tag=f"q2s{s}", name=f"q2s{s}")
               for s in range(NSP)]
        # k2z[h][s]: only rows h*D..h*D+63 live, rest zero — score matmuls
        # contract over full K=128 (keeps the PE HAM activity monitor warm)
        k2zs = [[cst.tile([P, SP], F32R, tag=f"k2z{h}_{s}", name=f"k2z{h}_{s}")
                 for s in range(NSP)] for h in range(HPC)]
        attn_t = [cst.tile([P, P], F32R, tag=f"attn{t}", name=f"attn{t}")
                  for t in range(NT)]
        vaug = [cst.tile([P, NT, D + 1], F32R, tag=f"vaug{h}", name=f"vaug{h}")
                for h in range(HPC)]
        ones_col = cst.tile([P, 1], F32)
        nc.vector.memset(ones_col[:], 1.0)
        zero_col = cst.tile([P, 1], F32)
        nc.vector.memset(zero_col[:], 0.0)
        for h in range(HPC):
            nc.vector.tensor_copy(vaug[h][:, :, D:D + 1],
                                  ones_col[:, 0:1, None].to_broadcast([P, NT, 1]))
            osl = slice((1 - h) * D, (2 - h) * D)   # the dead half of k2z[h]
            for s in range(NSP):
                nc.vector.tensor_copy(k2zs[h][s][osl, :],
                                      zero_col[osl, 0:1].to_broadcast([D, SP]))

        # weights
        wq_sb = cst.tile([P, KT, CW], F32R)
        nc.sync.dma_start(wq_sb[:], wqT.rearrange("(k p) m -> p k m", p=P))
        wk_sb = cst.tile([P, KT, CW], F32R)
        nc.sync.dma_start(wk_sb[:], wkT.rearrange("(k p) m -> p k m", p=P))
        wv_sb = cst.tile([P, KT, CW], F32R)
        nc.sync.dma_start(wv_sb[:], wvT.rearrange("(k p) m -> p k m", p=P))
        wo_sb = cst.tile([P, OUT_F], F32R)
        nc.sync.dma_start(wo_sb[:], woT[:])
        bq_sb = cst.tile([P, 1], F32)
        nc.sync.dma_start(bq_sb[:], bq[:, None])
        bk_sb = cst.tile([P, 1], F32)
        nc.sync.dma_start(bk_sb[:], bk[:, None])
        bv_sb = cst.tile([P, 1], F32)
        nc.sync.dma_start(bv_sb[:], bv[:, None])

        # ---- emission helpers ----
        def emit_proj_strip(s):
            xs = lp.tile([P, KT, SP], F32R, tag="xs", name="xs")
            nc.sync.dma_start(
                xs[:], xT.rearrange("(k p) n -> p k n", p=P)[:, :, s * SP:(s + 1) * SP])
            ps = pp.tile([P, SQ], F32, tag="ps_s", name="ps_q")
            for k in range(KT):
                nc.tensor.matmul(ps[:, 0:SP], wq_sb[:, k, :], xs[:, k, :],
                                 start=(k == 0), stop=(k == KT - 1))
            nc.vector.tensor_scalar_add(q2s[s][:], ps[:, 0:SP], bq_sb[:, 0:1])
            ps = pp.tile([P, SQ], F32, tag="ps_s", name="ps_k")
            for k in range(KT):
                nc.tensor.matmul(ps[:, 0:SP], wk_sb[:, k, :], xs[:, k, :],
                                 start=(k == 0), stop=(k == KT - 1))
            for h in range(HPC):
                hsl = slice(h * D, (h + 1) * D)
                nc.vector.tensor_scalar_add(k2zs[h][s][hsl, :], ps[hsl, 0:SP],
                                            bk_sb[hsl, 0:1])
            # v: project then transpose into vaug
            ps = pp.tile([P, SQ], F32, tag="ps_s", name="ps_v")
            for k in range(KT):
                nc.tensor.matmul(ps[:, 0:SP], wv_sb[:, k, :], xs[:, k, :],
                                 start=(k == 0), stop=(k == KT - 1))
            v2Ts = lp.tile([P, SP], F32, tag="v2Ts", name="v2Ts")
            nc.vector.tensor_scalar_add(v2Ts[:], ps[:, 0:SP], bv_sb[:, 0:1])
            for b in range(SP // P):
                t = s * (SP // P) + b
                ps_t = pp.tile([P, SQ], F32, tag="ps_s", name="ps_t")
                nc.tensor.transpose(ps_t[:, 0:P], v2Ts[:, b * P:(b + 1) * P],
                                    ident[:])
                for h in range(HPC):
                    nc.vector.tensor_copy(vaug[h][:, t, 0:D],
                                          ps_t[:, h * D:h * D + D])

        def emit_pv(ps_o, h, mk, p):
            for half in range(SQ // SP):
                fsl = slice(half * SP, (half + 1) * SP)
                nc.tensor.matmul(
                    ps_o[h][:, fsl],
                    vaug[h][:, mk, :],
                    p[:, fsl],
                    start=(mk == 0), stop=(mk == NT - 1),
                )

        def emit_attn_tiles(sq, ps_o, mks, pend):
            for mk in mks:
                use_pe = (mk % 2 == 0) or MASK_ALL_PE
                if mk % 2 == 0:
                    mt = wp.tile([P, SQ], F16, tag="mta", name="mta")
                    nc.sync.dma_start(
                        mt[:], maskA[(mk // 2) * P:(mk // 2 + 1) * P,
                                     sq * SQ:(sq + 1) * SQ])
                else:
                    mt = wp.tile([P, SQ], F16, tag="mtm", name="mtm")
                    nc.sync.dma_start(
                        mt[:], maskM[(mk // 2) * P:(mk // 2 + 1) * P,
                                     sq * SQ:(sq + 1) * SQ])
                for h in range(HPC):
                    ps_s = pp.tile([P, SQ], F32, tag="ps_s", name="ps_s")
                    for half in range(SQ // SP):
                        fsl = slice(half * SP, (half + 1) * SP)
                        nc.tensor.matmul(
                            ps_s[:, fsl],
                            k2zs[h][mk // 4][:, (mk % 4) * P:(mk % 4 + 1) * P],
                            q2s[sq * (SQ // SP) + half][:],
                            start=True, stop=not use_pe,
                        )
                        if use_pe:
                            nc.tensor.matmul(
                                ps_s[:, fsl], identH[:], mt[:, fsl],
                                start=False, stop=True,
                            )
                    p = wp.tile([P, SQ], F32R, tag="p", name="p")
                    nc.scalar.activation(p[:], ps_s[:], AF.Exp, scale=SCALE)
                    if not use_pe:
                        pm = wp.tile([P, SQ], F32R, tag="pm", name="pm")
                        nc.vector.tensor_tensor(pm[:], p[:], mt[:], ALU.mult)
                        p = pm
                    # software-pipeline: defer this tile's PV until after the
                    # next tile's scores so the PE stream never head-of-line
                    # blocks on the exp
                    pend.append((h, mk, p))
                    if len(pend) > 1:
                        emit_pv(ps_o, *pend.pop(0))

        def emit_epilogue(sq, ps_o):
            # stage PSUM accumulators to SBUF immediately so the next strip's
            # PV matmuls get the banks back as early as possible
            osb, bcs = [], []
            for h in range(HPC):
                ob = ep.tile([D + 1, SQ], F32, tag=f"osb{h}", name=f"osb{h}")
                nc.vector.tensor_copy(ob[:], ps_o[h][:])
                osb.append(ob)
            for h in range(HPC):
                zrow = ep.tile([1, SQ], F32, tag="zrow", name=f"zrow{h}")
                nc.vector.tensor_copy(zrow[:], osb[h][D:D + 1, :])
                recip = ep.tile([1, SQ], F32, tag="recip", name=f"recip{h}")
                nc.vector.reciprocal_approx_fast(recip[:], zrow[:])
                bc = ep.tile([D, SQ], F32, tag=f"bc{h}", name=f"bc{h}")
                nc.gpsimd.partition_broadcast(bc[:], recip[:])
                bcs.append(bc)
            # normalize per node tile so out_proj starts early
            for b in range(SQ // P):
                t = sq * (SQ // P) + b
                for h in range(HPC):
                    nc.vector.tensor_tensor(
                        attn_t[t][h * D:(h + 1) * D, :],
                        osb[h][0:D, b * P:(b + 1) * P],
                        bcs[h][:, b * P:(b + 1) * P], ALU.mult)
                ps_y = pp.tile([P, SQ], F32, tag="ps_s", name="ps_y")
                for f in range(OUT_F // 512):
                    nc.tensor.matmul(ps_y[:, f * 512:(f + 1) * 512],
                                     attn_t[t][:, :],
                                     wo_sb[:, f * 512:(f + 1) * 512],
                                     start=True, stop=True)
                ys = wp.tile([P, OUT_F], F32, tag="ys", name="ys")
                if b % 2 == 0:
                    nc.vector.tensor_copy(ys[:], ps_y[:])
                else:
                    nc.scalar.copy(ys[:], ps_y[:])
                nc.sync.dma_start(ypart[t * P:(t + 1) * P, :], ys[:])

        # ---- interleaved emission: A strips feed B(sq=0) chunks ----
        emit_proj_strip(0)
        emit_proj_strip(1)
        ps_o0 = [pso.tile([D + 1, SQ], F32, tag=f"ps_o{h}", name=f"ps_o{h}")
                 for h in range(HPC)]
        pend0 = []
        emit_attn_tiles(0, ps_o0, range(0, 8), pend0)
        emit_proj_strip(2)
        emit_attn_tiles(0, ps_o0, range(8, 12), pend0)
        emit_proj_strip(3)
        emit_attn_tiles(0, ps_o0, range(12, 16), pend0)
        emit_proj_strip(4)
        emit_attn_tiles(0, ps_o0, range(16, 20), pend0)
        emit_proj_strip(5)
        emit_attn_tiles(0, ps_o0, range(20, 24), pend0)
        ps_o_cur, pend_cur = ps_o0, pend0
        for sq in range(1, NSQ):
            ps_o_nxt = [pso.tile([D + 1, SQ], F32, tag=f"ps_o{h}",
                                 name=f"ps_o{h}_{sq}") for h in range(HPC)]
            pend_nxt = []
            emit_attn_tiles(sq, ps_o_nxt, range(0, 4), pend_nxt)
            for args in pend_cur:
                emit_pv(ps_o_cur, *args)
            emit_epilogue(sq - 1, ps_o_cur)
            emit_attn_tiles(sq, ps_o_nxt, range(4, NT), pend_nxt)
            ps_o_cur, pend_cur = ps_o_nxt, pend_nxt
        for args in pend_cur:
            emit_pv(ps_o_cur, *args)
        emit_epilogue(NSQ - 1, ps_o_cur)

    nc.compile()
    return nc


_PROGRAM = None
LAST_RESULTS = None


def _get_program():
    global _PROGRAM
    if _PROGRAM is None:
        _PROGRAM = build_program()
    return _PROGRAM


def _softplus(x):
    x = np.asarray(x, np.float32)
    return np.logaddexp(0.0, x).astype(np.float32)


def host_prep(inputs):
    x = np.asarray(inputs["x"], np.float32)
    edge_index = np.asarray(inputs["edge_index"])
    edge_type = np.asarray(inputs["edge_type"])
    etw = np.asarray(inputs["edge_type_weights"], np.float32)

    def f32(k):
        return np.asarray(inputs[k], np.float32)

    # compose the two linear layers: q2 = x @ (wiq@wq).T + (wiq@bq + biq)
    WQ = f32("wiq") @ f32("wq")
    bQ = f32("wiq") @ f32("bq") + f32("biq")
    WK = f32("wik") @ f32("wk")
    bK = f32("wik") @ f32("bk") + f32("bik")
    WV = f32("wiv") @ f32("wv")
    bV = f32("wiv") @ f32("bv") + f32("biv")
    wo = f32("wo")
    bo = f32("bo")

    # multiplicative mask, transposed: maskT[m, n] = exp(add_mask[n, m])
    w = _softplus(etw)
    NEG = np.float32(-60000.0)
    M = np.full((N, N), NEG, dtype=np.float32)
    src, dst = edge_index[0], edge_index[1]
    wv8 = (w * np.float32(1.0 / SCALE)).astype(np.float32)
    M[src, dst] = wv8[edge_type - 1]           # last write wins, like jax .at[].set
    diag = np.diagonal(M).copy()
    didx = np.arange(N)
    M[didx, didx] = np.where(diag == NEG, wv8[3], diag)
    MT = np.ascontiguousarray(M.T)             # [key m, query n], additive * 8
    # even key tiles use the additive form on the PE, odd tiles the
    # multiplicative exp-form on the DVE
    MT4 = MT.reshape(NT, P, N)
    maskA = MT4[0::2].reshape(N // 2, N).astype(np.float16)
    if MASK_ALL_PE:
        maskM = MT4[1::2].reshape(N // 2, N).astype(np.float16)
    else:
        maskM = np.exp(MT4[1::2].reshape(N // 2, N).astype(np.float64)
                       * np.float64(SCALE)).astype(np.float16)

    xT = np.ascontiguousarray(x.T)

    in_maps = []
    for c in range(NCORES):
        rs = slice(c * CW, (c + 1) * CW)
        in_maps.append({
            "xT": xT,
            "maskA": maskA,
            "maskM": maskM,
            "wqT": np.ascontiguousarray(WQ[rs].T),
            "wkT": np.ascontiguousarray(WK[rs].T),
            "wvT": np.ascontiguousarray(WV[rs].T),
            "bq": np.ascontiguousarray(bQ[rs]),
            "bk": np.ascontiguousarray(bK[rs]),
            "bv": np.ascontiguousarray(bV[rs]),
            "woT": np.ascontiguousarray(wo[:, rs].T),
        })
    return in_maps, bo


def kernel(**inputs) -> np.ndarray:
    global LAST_RESULTS
    in_maps, bo = host_prep(inputs)
    nc = _get_program()
    trace = bool(os.environ.get("KERNEL_TRACE"))
    res = run_bass_kernel_spmd(nc, in_maps, list(range(NCORES)), trace=trace)
    LAST_RESULTS = res
    y = bo[None, :].astype(np.float32).repeat(N, axis=0)
    for c in range(NCORES):
        y += res.results[c]["ypart"]
    return y


# revision 31
# speedup vs baseline: 1.1987x; 1.0587x over previous
"""Sparse multi-head attention (nn_MultiHeadAttention_44332652429419) on 8 trn2 cores.

Strategy (tensor-parallel over H=16 heads, 2 heads per core):
  Host: compose the two stacked linear layers (q/k/v_proj followed by
        MultiheadAttention in_proj) into one weight per tensor; build the
        dense multiplicative mask exp(additive_mask) transposed; transpose x.
  Device (per core, SPMD with per-core weight slices):
    q2T/k2T/v2T = W_c @ x.T + b_c           [128, 3072] (2 heads x 64 dims)
    scoresT[mk,nq] = k2T_h.T-slice @ q2T_h  (K=64, two heads row-packed in PE)
    P = exp(scoresT * 1/8) * maskT          (ACT exp from PSUM, DVE multiply)
    outT_aug = [v_h | 1].T @ P              (rowsum via ones-augmented V)
    attnT = outT[:64] / outT[64]            (DVE recip + partition broadcast)
    ypart = attnT.T-slices @ woT_c          (partial out_proj, K=128)
  Host: y = sum_c ypart_c + bo
"""
import os
import sys

sys.path.insert(0, "/opt/trn_rl_repo")

import numpy as np
from contextlib import ExitStack

import concourse.bass as bass
import concourse.bacc as bacc
import concourse.mybir as mybir
import concourse.tile as tile
from concourse.bass_utils import run_bass_kernel_spmd
from concourse.masks import make_identity

F32 = mybir.dt.float32
F32R = mybir.dt.float32r
BF16 = mybir.dt.bfloat16
F16 = mybir.dt.float16
AF = mybir.ActivationFunctionType
ALU = mybir.AluOpType

N = 3072
IN_F = 1024
OUT_F = 1024
H = 16
D = 64
NCORES = 8
HPC = H // NCORES            # heads per core = 2
CW = HPC * D                 # per-core width = 128
P = 128
NT = N // P                  # 24 node tiles
KT = IN_F // P               # 8 contraction tiles
SQ = 1024                    # query strip width (phase B)
NSQ = N // SQ                # 3 strips
SP = 512                     # proj strip width (phase A)
NSP = N // SP                # 6 strips
SCALE = 1.0 / 8.0            # 1/sqrt(D)

MASK_DT = F32R               # additive mask, pre-scaled by 1/SCALE
MASK_ALL_PE = False          # apply mask via PE identity-add for all tiles


def build_program():
    nc = bacc.Bacc()
    xT = nc.declare_dram_parameter("xT", [IN_F, N], F32R, isOutput=False)
    # additive mask (pre-scaled by 1/SCALE) for even key tiles, multiplicative
    # exp-mask for odd key tiles — hybrid PE/DVE mask application
    maskA = nc.declare_dram_parameter("maskA", [N // 2, N], F16, isOutput=False)
    maskM = nc.declare_dram_parameter("maskM", [N // 2, N], F16, isOutput=False)
    wqT = nc.declare_dram_parameter("wqT", [IN_F, CW], F32R, isOutput=False)
    wkT = nc.declare_dram_parameter("wkT", [IN_F, CW], F32R, isOutput=False)
    wvT = nc.declare_dram_parameter("wvT", [IN_F, CW], F32R, isOutput=False)
    bq = nc.declare_dram_parameter("bq", [CW], F32, isOutput=False)
    bk = nc.declare_dram_parameter("bk", [CW], F32, isOutput=False)
    bv = nc.declare_dram_parameter("bv", [CW], F32, isOutput=False)
    woT = nc.declare_dram_parameter("woT", [CW, OUT_F], F32R, isOutput=False)
    ypart = nc.declare_dram_parameter("ypart", [N, OUT_F], F32, isOutput=True)

    with tile.TileContext(nc) as tc, ExitStack() as ctx:
        cst = ctx.enter_context(tc.tile_pool(name="cst", bufs=1))
        lp = ctx.enter_context(tc.tile_pool(name="lp", bufs=2))       # xs/v2Ts
        wp = ctx.enter_context(tc.tile_pool(name="wp", bufs=4))       # loop tiles
        ep = ctx.enter_context(tc.tile_pool(name="ep", bufs=1))       # epilogue
        pp = ctx.enter_context(tc.tile_pool(name="pp", bufs=2, space="PSUM"))
        pso = ctx.enter_context(tc.tile_pool(name="pso", bufs=1, space="PSUM"))

        ident = cst.tile([P, P], F32)
        make_identity(nc, ident)
        identR = cst.tile([P, P], F32R)
        nc.vector.tensor_copy(identR[:], ident[:])
        identH = cst.tile([P, P], F16)
        nc.vector.tensor_copy(identH[:], ident[:])

        # per-512-strip persistent tensors (fine-grained deps let phase B
        # start while projections still run)
        q2s = [cst.tile([P, SP], F32R, tag=f"q2s{s}", name=f"q2s{s}")
               for s in range(NSP)]
        # k2z[h][s]: only rows h*D..h*D+63 live, rest zero — score matmuls
        # contract over full K=128 (keeps the PE HAM activity monitor warm)
        k2zs = [[cst.tile([P, SP], F32R, tag=f"k2z{h}_{s}", name=f"k2z{h}_{s}")
                 for s in range(NSP)] for h in range(HPC)]
        attn_t = [cst.tile([P, P], F32R, tag=f"attn{t}", name=f"attn{t}")
                  for t in range(NT)]
        vaug = [cst.tile([P, NT, D + 1], F32R, tag=f"vaug{h}", name=f"vaug{h}")
                for h in range(HPC)]
        ones_col = cst.tile([P, 1], F32)
        nc.vector.memset(ones_col[:], 1.0)
        zero_col = cst.tile([P, 1], F32)
        nc.vector.memset(zero_col[:], 0.0)
        for h in range(HPC):
            nc.vector.tensor_copy(vaug[h][:, :, D:D + 1],
                                  ones_col[:, 0:1, None].to_broadcast([P, NT, 1]))
            osl = slice((1 - h) * D, (2 - h) * D)   # the dead half of k2z[h]
            for s in range(NSP):
                nc.vector.tensor_copy(k2zs[h][s][osl, :],
                                      zero_col[osl, 0:1].to_broadcast([D, SP]))

        # weights
        wq_sb = cst.tile([P, KT, CW], F32R)
        nc.sync.dma_start(wq_sb[:], wqT.rearrange("(k p) m -> p k m", p=P))
        wk_sb = cst.tile([P, KT, CW], F32R)
        nc.sync.dma_start(wk_sb[:], wkT.rearrange("(k p) m -> p k m", p=P))
        wv_sb = cst.tile([P, KT, CW], F32R)
        nc.sync.dma_start(wv_sb[:], wvT.rearrange("(k p) m -> p k m", p=P))
        wo_sb = cst.tile([P, OUT_F], F32R)
        nc.sync.dma_start(wo_sb[:], woT[:])
        bq_sb = cst.tile([P, 1], F32)
        nc.sync.dma_start(bq_sb[:], bq[:, None])
        bk_sb = cst.tile([P, 1], F32)
        nc.sync.dma_start(bk_sb[:], bk[:, None])
        bv_sb = cst.tile([P, 1], F32)
        nc.sync.dma_start(bv_sb[:], bv[:, None])

        # ---- emission helpers ----
        def emit_proj_strip(s):
            xs = lp.tile([P, KT, SP], F32R, tag="xs", name="xs")
            nc.sync.dma_start(
                xs[:], xT.rearrange("(k p) n -> p k n", p=P)[:, :, s * SP:(s + 1) * SP])
            ps = pp.tile([P, SQ], F32, tag="ps_s", name="ps_q")
            for k in range(KT):
                nc.tensor.matmul(ps[:, 0:SP], wq_sb[:, k, :], xs[:, k, :],
                                 start=(k == 0), stop=(k == KT - 1))
            nc.vector.tensor_scalar_add(q2s[s][:], ps[:, 0:SP], bq_sb[:, 0:1])
            ps = pp.tile([P, SQ], F32, tag="ps_s", name="ps_k")
            for k in range(KT):
                nc.tensor.matmul(ps[:, 0:SP], wk_sb[:, k, :], xs[:, k, :],
                                 start=(k == 0), stop=(k == KT - 1))
            for h in range(HPC):
                hsl = slice(h * D, (h + 1) * D)
                nc.vector.tensor_scalar_add(k2zs[h][s][hsl, :], ps[hsl, 0:SP],
                                            bk_sb[hsl, 0:1])
            # v: project then transpose into vaug
            ps = pp.tile([P, SQ], F32, tag="ps_s", name="ps_v")
            for k in range(KT):
                nc.tensor.matmul(ps[:, 0:SP], wv_sb[:, k, :], xs[:, k, :],
                                 start=(k == 0), stop=(k == KT - 1))
            v2Ts = lp.tile([P, SP], F32, tag="v2Ts", name="v2Ts")
            nc.vector.tensor_scalar_add(v2Ts[:], ps[:, 0:SP], bv_sb[:, 0:1])
            for b in range(SP // P):
                t = s * (SP // P) + b
                ps_t = pp.tile([P, SQ], F32, tag="ps_s", name="ps_t")
                nc.tensor.transpose(ps_t[:, 0:P], v2Ts[:, b * P:(b + 1) * P],
                                    ident[:])
                for h in range(HPC):
                    nc.vector.tensor_copy(vaug[h][:, t, 0:D],
                                          ps_t[:, h * D:h * D + D])

        def emit_pv(ps_o, h, mk, p):
            for half in range(SQ // SP):
                fsl = slice(half * SP, (half + 1) * SP)
                nc.tensor.matmul(
                    ps_o[h][:, fsl],
                    vaug[h][:, mk, :],
                    p[:, fsl],
                    start=(mk == 0), stop=(mk == NT - 1),
                )

        def emit_attn_tiles(sq, ps_o, mks, pend):
            for mk in mks:
                use_pe = (mk % 2 == 0) or MASK_ALL_PE
                if mk % 2 == 0:
                    mt = wp.tile([P, SQ], F16, tag="mta", name="mta")
                    nc.sync.dma_start(
                        mt[:], maskA[(mk // 2) * P:(mk // 2 + 1) * P,
                                     sq * SQ:(sq + 1) * SQ])
                else:
                    mt = wp.tile([P, SQ], F16, tag="mtm", name="mtm")
                    nc.sync.dma_start(
                        mt[:], maskM[(mk // 2) * P:(mk // 2 + 1) * P,
                                     sq * SQ:(sq + 1) * SQ])
                for h in range(HPC):
                    ps_s = pp.tile([P, SQ], F32, tag="ps_s", name="ps_s")
                    for half in range(SQ // SP):
                        fsl = slice(half * SP, (half + 1) * SP)
                        nc.tensor.matmul(
                            ps_s[:, fsl],
                            k2zs[h][mk // 4][:, (mk % 4) * P:(mk % 4 + 1) * P],
                            q2s[sq * (SQ // SP) + half][:],
                            start=True, stop=not use_pe,
                        )
                        if use_pe:
                            nc.tensor.matmul(
                                ps_s[:, fsl], identH[:], mt[:, fsl],
                                start=False, stop=True,
                            )
                    p = wp.tile([P, SQ], F32R, tag="p", name="p")
                    nc.scalar.activation(p[:], ps_s[:], AF.Exp, scale=SCALE)
                    if not use_pe:
                        pm = wp.tile([P, SQ], F32R, tag="pm", name="pm")
                        nc.vector.tensor_tensor(pm[:], p[:], mt[:], ALU.mult)
                        p = pm
                    # software-pipeline: defer this tile's PV until after the
                    # next tile's scores so the PE stream never head-of-line
                    # blocks on the exp
                    pend.append((h, mk, p))
                    if len(pend) > 2:
                        emit_pv(ps_o, *pend.pop(0))

        def emit_epilogue(sq, ps_o):
            # stage PSUM accumulators to SBUF immediately so the next strip's
            # PV matmuls get the banks back as early as possible
            osb, bcs = [], []
            for h in range(HPC):
                ob = ep.tile([D + 1, SQ], F32, tag=f"osb{h}", name=f"osb{h}")
                nc.vector.tensor_copy(ob[:], ps_o[h][:])
                osb.append(ob)
            for h in range(HPC):
                zrow = ep.tile([1, SQ], F32, tag="zrow", name=f"zrow{h}")
                nc.vector.tensor_copy(zrow[:], osb[h][D:D + 1, :])
                recip = ep.tile([1, SQ], F32, tag="recip", name=f"recip{h}")
                nc.vector.reciprocal_approx_fast(recip[:], zrow[:])
                bc = ep.tile([D, SQ], F32, tag=f"bc{h}", name=f"bc{h}")
                nc.gpsimd.partition_broadcast(bc[:], recip[:])
                bcs.append(bc)
            # normalize per node tile so out_proj starts early
            for b in range(SQ // P):
                t = sq * (SQ // P) + b
                for h in range(HPC):
                    nc.vector.tensor_tensor(
                        attn_t[t][h * D:(h + 1) * D, :],
                        osb[h][0:D, b * P:(b + 1) * P],
                        bcs[h][:, b * P:(b + 1) * P], ALU.mult)
                ps_y = pp.tile([P, SQ], F32, tag="ps_s", name="ps_y")
                for f in range(OUT_F // 512):
                    nc.tensor.matmul(ps_y[:, f * 512:(f + 1) * 512],
                                     attn_t[t][:, :],
                                     wo_sb[:, f * 512:(f + 1) * 512],
                                     start=True, stop=True)
                ys = wp.tile([P, OUT_F], F32, tag="ys", name="ys")
                if b % 2 == 0:
                    nc.vector.tensor_copy(ys[:], ps_y[:])
                else:
                    nc.scalar.copy(ys[:], ps_y[:])
                nc.sync.dma_start(ypart[t * P:(t + 1) * P, :], ys[:])

        # ---- interleaved emission: A strips feed B(sq=0) chunks ----
        emit_proj_strip(0)
        emit_proj_strip(1)
        ps_o0 = [pso.tile([D + 1, SQ], F32, tag=f"ps_o{h}", name=f"ps_o{h}")
                 for h in range(HPC)]
        pend0 = []
        emit_attn_tiles(0, ps_o0, range(0, 8), pend0)
        emit_proj_strip(2)
        emit_attn_tiles(0, ps_o0, range(8, 12), pend0)
        emit_proj_strip(3)
        emit_attn_tiles(0, ps_o0, range(12, 16), pend0)
        emit_proj_strip(4)
        emit_attn_tiles(0, ps_o0, range(16, 20), pend0)
        emit_proj_strip(5)
        emit_attn_tiles(0, ps_o0, range(20, 24), pend0)
        ps_o_cur, pend_cur = ps_o0, pend0
        for sq in range(1, NSQ):
            ps_o_nxt = [pso.tile([D + 1, SQ], F32, tag=f"ps_o{h}",
                                 name=f"ps_o{h}_{sq}") for h in range(HPC)]
            pend_nxt = []
            emit_attn_tiles(sq, ps_o_nxt, range(0, 4), pend_nxt)
            for args in pend_cur:
                emit_pv(ps_o_cur, *args)
            emit_epilogue(sq - 1, ps_o_cur)
            emit_attn_tiles(sq, ps_o_nxt, range(4, NT), pend_nxt)
            ps_o_cur, pend_cur = ps_o_nxt, pend_nxt
        for args in pend_cur:
            emit_pv(ps_o_cur, *args)
        emit_epilogue(NSQ - 1, ps_o_cur)

    nc.compile()
    return nc


_PROGRAM = None
LAST_RESULTS = None


def _get_program():
    global _PROGRAM
    if _PROGRAM is None:
        _PROGRAM = build_program()
    return _PROGRAM


def _softplus(x):
    x = np.asarray(x, np.float32)
    return np.logaddexp(0.0, x).astype(np.float32)


def host_prep(inputs):
    x = np.asarray(inputs["x"], np.float32)
    edge_index = np.asarray(inputs["edge_index"])
    edge_type = np.asarray(inputs["edge_type"])
    etw = np.asarray(inputs["edge_type_weights"], np.float32)

    def f32(k):
        return np.asarray(inputs[k], np.float32)

    # compose the two linear layers: q2 = x @ (wiq@wq).T + (wiq@bq + biq)
    WQ = f32("wiq") @ f32("wq")
    bQ = f32("wiq") @ f32("bq") + f32("biq")
    WK = f32("wik") @ f32("wk")
    bK = f32("wik") @ f32("bk") + f32("bik")
    WV = f32("wiv") @ f32("wv")
    bV = f32("wiv") @ f32("bv") + f32("biv")
    wo = f32("wo")
    bo = f32("bo")

    # multiplicative mask, transposed: maskT[m, n] = exp(add_mask[n, m])
    w = _softplus(etw)
    NEG = np.float32(-60000.0)
    M = np.full((N, N), NEG, dtype=np.float32)
    src, dst = edge_index[0], edge_index[1]
    wv8 = (w * np.float32(1.0 / SCALE)).astype(np.float32)
    M[src, dst] = wv8[edge_type - 1]           # last write wins, like jax .at[].set
    diag = np.diagonal(M).copy()
    didx = np.arange(N)
    M[didx, didx] = np.where(diag == NEG, wv8[3], diag)
    MT = np.ascontiguousarray(M.T)             # [key m, query n], additive * 8
    # even key tiles use the additive form on the PE, odd tiles the
    # multiplicative exp-form on the DVE
    MT4 = MT.reshape(NT, P, N)
    maskA = MT4[0::2].reshape(N // 2, N).astype(np.float16)
    if MASK_ALL_PE:
        maskM = MT4[1::2].reshape(N // 2, N).astype(np.float16)
    else:
        maskM = np.exp(MT4[1::2].reshape(N // 2, N).astype(np.float64)
                       * np.float64(SCALE)).astype(np.float16)

    xT = np.ascontiguousarray(x.T)

    in_maps = []
    for c in range(NCORES):
        rs = slice(c * CW, (c + 1) * CW)
        in_maps.append({
            "xT": xT,
            "maskA": maskA,
            "maskM": maskM,
            "wqT": np.ascontiguousarray(WQ[rs].T),
            "wkT": np.ascontiguousarray(WK[rs].T),
            "wvT": np.ascontiguousarray(WV[rs].T),
            "bq": np.ascontiguousarray(bQ[rs]),
            "bk": np.ascontiguousarray(bK[rs]),
            "bv": np.ascontiguousarray(bV[rs]),
            "woT": np.ascontiguousarray(wo[:, rs].T),
        })
    return in_maps, bo


def kernel(**inputs) -> np.ndarray:
    global LAST_RESULTS
    in_maps, bo = host_prep(inputs)
    nc = _get_program()
    trace = bool(os.environ.get("KERNEL_TRACE"))
    res = run_bass_kernel_spmd(nc, in_maps, list(range(NCORES)), trace=trace)
    LAST_RESULTS = res
    y = bo[None, :].astype(np.float32).repeat(N, axis=0)
    for c in range(NCORES):
        y += res.results[c]["ypart"]
    return y


# revision 32
# speedup vs baseline: 1.2076x; 1.0074x over previous
"""Sparse multi-head attention (nn_MultiHeadAttention_44332652429419) on 8 trn2 cores.

Strategy (tensor-parallel over H=16 heads, 2 heads per core):
  Host: compose the two stacked linear layers (q/k/v_proj followed by
        MultiheadAttention in_proj) into one weight per tensor; build the
        dense multiplicative mask exp(additive_mask) transposed; transpose x.
  Device (per core, SPMD with per-core weight slices):
    q2T/k2T/v2T = W_c @ x.T + b_c           [128, 3072] (2 heads x 64 dims)
    scoresT[mk,nq] = k2T_h.T-slice @ q2T_h  (K=64, two heads row-packed in PE)
    P = exp(scoresT * 1/8) * maskT          (ACT exp from PSUM, DVE multiply)
    outT_aug = [v_h | 1].T @ P              (rowsum via ones-augmented V)
    attnT = outT[:64] / outT[64]            (DVE recip + partition broadcast)
    ypart = attnT.T-slices @ woT_c          (partial out_proj, K=128)
  Host: y = sum_c ypart_c + bo
"""
import os
import sys

sys.path.insert(0, "/opt/trn_rl_repo")

import numpy as np
from contextlib import ExitStack

import concourse.bass as bass
import concourse.bacc as bacc
import concourse.mybir as mybir
import concourse.tile as tile
from concourse.bass_utils import run_bass_kernel_spmd
from concourse.masks import make_identity

F32 = mybir.dt.float32
F32R = mybir.dt.float32r
BF16 = mybir.dt.bfloat16
F16 = mybir.dt.float16
AF = mybir.ActivationFunctionType
ALU = mybir.AluOpType

N = 3072
IN_F = 1024
OUT_F = 1024
H = 16
D = 64
NCORES = 8
HPC = H // NCORES            # heads per core = 2
CW = HPC * D                 # per-core width = 128
P = 128
NT = N // P                  # 24 node tiles
KT = IN_F // P               # 8 contraction tiles
SQ = 1024                    # query strip width (phase B)
NSQ = N // SQ                # 3 strips
SP = 512                     # proj strip width (phase A)
NSP = N // SP                # 6 strips
SCALE = 1.0 / 8.0            # 1/sqrt(D)

MASK_DT = F32R               # additive mask, pre-scaled by 1/SCALE
MASK_ALL_PE = False          # apply mask via PE identity-add for all tiles


def build_program():
    nc = bacc.Bacc()
    xT = nc.declare_dram_parameter("xT", [IN_F, N], F32R, isOutput=False)
    # additive mask (pre-scaled by 1/SCALE) for even key tiles, multiplicative
    # exp-mask for odd key tiles — hybrid PE/DVE mask application
    maskA = nc.declare_dram_parameter("maskA", [N // 2, N], F16, isOutput=False)
    maskM = nc.declare_dram_parameter("maskM", [N // 2, N], F16, isOutput=False)
    wqT = nc.declare_dram_parameter("wqT", [IN_F, CW], F32R, isOutput=False)
    wkT = nc.declare_dram_parameter("wkT", [IN_F, CW], F32R, isOutput=False)
    wvT = nc.declare_dram_parameter("wvT", [IN_F, CW], F32R, isOutput=False)
    bq = nc.declare_dram_parameter("bq", [CW], F32, isOutput=False)
    bk = nc.declare_dram_parameter("bk", [CW], F32, isOutput=False)
    bv = nc.declare_dram_parameter("bv", [CW], F32, isOutput=False)
    woT = nc.declare_dram_parameter("woT", [CW, OUT_F], F32R, isOutput=False)
    ypart = nc.declare_dram_parameter("ypart", [N, OUT_F], F32, isOutput=True)

    with tile.TileContext(nc) as tc, ExitStack() as ctx:
        cst = ctx.enter_context(tc.tile_pool(name="cst", bufs=1))
        lp = ctx.enter_context(tc.tile_pool(name="lp", bufs=2))       # xs/v2Ts
        wp = ctx.enter_context(tc.tile_pool(name="wp", bufs=4))       # loop tiles
        ep = ctx.enter_context(tc.tile_pool(name="ep", bufs=1))       # epilogue
        pp = ctx.enter_context(tc.tile_pool(name="pp", bufs=2, space="PSUM"))
        pso = ctx.enter_context(tc.tile_pool(name="pso", bufs=1, space="PSUM"))

        ident = cst.tile([P, P], F32)
        make_identity(nc, ident)
        identR = cst.tile([P, P], F32R)
        nc.vector.tensor_copy(identR[:], ident[:])
        identH = cst.tile([P, P], F16)
        nc.vector.tensor_copy(identH[:], ident[:])

        # per-512-strip persistent tensors (fine-grained deps let phase B
        # start while projections still run)
        q2s = [cst.tile([P, SP], F32R, tag=f"q2s{s}", name=f"q2s{s}")
               for s in range(NSP)]
        # k2z[h][s]: only rows h*D..h*D+63 live, rest zero — score matmuls
        # contract over full K=128 (keeps the PE HAM activity monitor warm)
        k2zs = [[cst.tile([P, SP], F32R, tag=f"k2z{h}_{s}", name=f"k2z{h}_{s}")
                 for s in range(NSP)] for h in range(HPC)]
        attn_t = [cst.tile([P, P], F32R, tag=f"attn{t}", name=f"attn{t}")
                  for t in range(NT)]
        vaug = [cst.tile([P, NT, D + 1], F32R, tag=f"vaug{h}", name=f"vaug{h}")
                for h in range(HPC)]
        ones_col = cst.tile([P, 1], F32)
        nc.vector.memset(ones_col[:], 1.0)
        zero_col = cst.tile([P, 1], F32)
        nc.vector.memset(zero_col[:], 0.0)
        for h in range(HPC):
            nc.vector.tensor_copy(vaug[h][:, :, D:D + 1],
                                  ones_col[:, 0:1, None].to_broadcast([P, NT, 1]))
            osl = slice((1 - h) * D, (2 - h) * D)   # the dead half of k2z[h]
            for s in range(NSP):
                nc.vector.tensor_copy(k2zs[h][s][osl, :],
                                      zero_col[osl, 0:1].to_broadcast([D, SP]))

        # weights
        wq_sb = cst.tile([P, KT, CW], F32R)
        nc.sync.dma_start(wq_sb[:], wqT.rearrange("(k p) m -> p k m", p=P))
        wk_sb = cst.tile([P, KT, CW], F32R)
        nc.sync.dma_start(wk_sb[:], wkT.rearrange("(k p) m -> p k m", p=P))
        wv_sb = cst.tile([P, KT, CW], F32R)
        nc.sync.dma_start(wv_sb[:], wvT.rearrange("(k p) m -> p k m", p=P))
        wo_sb = cst.tile([P, OUT_F], F32R)
        nc.sync.dma_start(wo_sb[:], woT[:])
        bq_sb = cst.tile([P, 1], F32)
        nc.sync.dma_start(bq_sb[:], bq[:, None])
        bk_sb = cst.tile([P, 1], F32)
        nc.sync.dma_start(bk_sb[:], bk[:, None])
        bv_sb = cst.tile([P, 1], F32)
        nc.sync.dma_start(bv_sb[:], bv[:, None])

        # ---- emission helpers ----
        def emit_proj_strip(s):
            xs = lp.tile([P, KT, SP], F32R, tag="xs", name="xs")
            nc.sync.dma_start(
                xs[:], xT.rearrange("(k p) n -> p k n", p=P)[:, :, s * SP:(s + 1) * SP])
            ps = pp.tile([P, SQ], F32, tag="ps_s", name="ps_q")
            for k in range(KT):
                nc.tensor.matmul(ps[:, 0:SP], wq_sb[:, k, :], xs[:, k, :],
                                 start=(k == 0), stop=(k == KT - 1))
            nc.vector.tensor_scalar_add(q2s[s][:], ps[:, 0:SP], bq_sb[:, 0:1])
            ps = pp.tile([P, SQ], F32, tag="ps_s", name="ps_k")
            for k in range(KT):
                nc.tensor.matmul(ps[:, 0:SP], wk_sb[:, k, :], xs[:, k, :],
                                 start=(k == 0), stop=(k == KT - 1))
            for h in range(HPC):
                hsl = slice(h * D, (h + 1) * D)
                nc.vector.tensor_scalar_add(k2zs[h][s][hsl, :], ps[hsl, 0:SP],
                                            bk_sb[hsl, 0:1])
            # v: project then transpose into vaug
            ps = pp.tile([P, SQ], F32, tag="ps_s", name="ps_v")
            for k in range(KT):
                nc.tensor.matmul(ps[:, 0:SP], wv_sb[:, k, :], xs[:, k, :],
                                 start=(k == 0), stop=(k == KT - 1))
            v2Ts = lp.tile([P, SP], F32, tag="v2Ts", name="v2Ts")
            nc.vector.tensor_scalar_add(v2Ts[:], ps[:, 0:SP], bv_sb[:, 0:1])
            for b in range(SP // P):
                t = s * (SP // P) + b
                ps_t = pp.tile([P, SQ], F32, tag="ps_s", name="ps_t")
                nc.tensor.transpose(ps_t[:, 0:P], v2Ts[:, b * P:(b + 1) * P],
                                    ident[:])
                for h in range(HPC):
                    nc.vector.tensor_copy(vaug[h][:, t, 0:D],
                                          ps_t[:, h * D:h * D + D])

        def emit_pv(ps_o, h, mk, p):
            for half in range(SQ // SP):
                fsl = slice(half * SP, (half + 1) * SP)
                nc.tensor.matmul(
                    ps_o[h][:, fsl],
                    vaug[h][:, mk, :],
                    p[:, fsl],
                    start=(mk == 0), stop=(mk == NT - 1),
                )

        def emit_attn_tiles(sq, ps_o, mks, pend):
            for mk in mks:
                use_pe = (mk % 2 == 0) or MASK_ALL_PE
                if mk % 2 == 0:
                    mt = wp.tile([P, SQ], F16, tag="mta", name="mta")
                    nc.sync.dma_start(
                        mt[:], maskA[(mk // 2) * P:(mk // 2 + 1) * P,
                                     sq * SQ:(sq + 1) * SQ])
                else:
                    mt = wp.tile([P, SQ], F16, tag="mtm", name="mtm")
                    nc.sync.dma_start(
                        mt[:], maskM[(mk // 2) * P:(mk // 2 + 1) * P,
                                     sq * SQ:(sq + 1) * SQ])
                for h in range(HPC):
                    ps_s = pp.tile([P, SQ], F32, tag="ps_s", name="ps_s")
                    for half in range(SQ // SP):
                        fsl = slice(half * SP, (half + 1) * SP)
                        nc.tensor.matmul(
                            ps_s[:, fsl],
                            k2zs[h][mk // 4][:, (mk % 4) * P:(mk % 4 + 1) * P],
                            q2s[sq * (SQ // SP) + half][:],
                            start=True, stop=not use_pe,
                        )
                        if use_pe:
                            nc.tensor.matmul(
                                ps_s[:, fsl], identH[:], mt[:, fsl],
                                start=False, stop=True,
                            )
                    p = wp.tile([P, SQ], F32R, tag="p", name="p")
                    nc.scalar.activation(p[:], ps_s[:], AF.Exp, scale=SCALE)
                    if not use_pe:
                        pm = wp.tile([P, SQ], F32R, tag="pm", name="pm")
                        nc.vector.tensor_tensor(pm[:], p[:], mt[:], ALU.mult)
                        p = pm
                    # software-pipeline: defer this tile's PV until after the
                    # next tile's scores so the PE stream never head-of-line
                    # blocks on the exp
                    pend.append((h, mk, p))
                    if len(pend) > 3:
                        emit_pv(ps_o, *pend.pop(0))

        def emit_epilogue(sq, ps_o):
            # stage PSUM accumulators to SBUF immediately so the next strip's
            # PV matmuls get the banks back as early as possible
            osb, bcs = [], []
            for h in range(HPC):
                ob = ep.tile([D + 1, SQ], F32, tag=f"osb{h}", name=f"osb{h}")
                nc.vector.tensor_copy(ob[:], ps_o[h][:])
                osb.append(ob)
            for h in range(HPC):
                zrow = ep.tile([1, SQ], F32, tag="zrow", name=f"zrow{h}")
                nc.vector.tensor_copy(zrow[:], osb[h][D:D + 1, :])
                recip = ep.tile([1, SQ], F32, tag="recip", name=f"recip{h}")
                nc.vector.reciprocal_approx_fast(recip[:], zrow[:])
                bc = ep.tile([D, SQ], F32, tag=f"bc{h}", name=f"bc{h}")
                nc.gpsimd.partition_broadcast(bc[:], recip[:])
                bcs.append(bc)
            # normalize per node tile so out_proj starts early
            for b in range(SQ // P):
                t = sq * (SQ // P) + b
                for h in range(HPC):
                    nc.vector.tensor_tensor(
                        attn_t[t][h * D:(h + 1) * D, :],
                        osb[h][0:D, b * P:(b + 1) * P],
                        bcs[h][:, b * P:(b + 1) * P], ALU.mult)
                ps_y = pp.tile([P, SQ], F32, tag="ps_s", name="ps_y")
                for f in range(OUT_F // 512):
                    nc.tensor.matmul(ps_y[:, f * 512:(f + 1) * 512],
                                     attn_t[t][:, :],
                                     wo_sb[:, f * 512:(f + 1) * 512],
                                     start=True, stop=True)
                ys = wp.tile([P, OUT_F], F32, tag="ys", name="ys")
                if b % 2 == 0:
                    nc.vector.tensor_copy(ys[:], ps_y[:])
                else:
                    nc.scalar.copy(ys[:], ps_y[:])
                nc.sync.dma_start(ypart[t * P:(t + 1) * P, :], ys[:])

        # ---- interleaved emission: A strips feed B(sq=0) chunks ----
        emit_proj_strip(0)
        emit_proj_strip(1)
        ps_o0 = [pso.tile([D + 1, SQ], F32, tag=f"ps_o{h}", name=f"ps_o{h}")
                 for h in range(HPC)]
        pend0 = []
        emit_attn_tiles(0, ps_o0, range(0, 8), pend0)
        emit_proj_strip(2)
        emit_attn_tiles(0, ps_o0, range(8, 12), pend0)
        emit_proj_strip(3)
        emit_attn_tiles(0, ps_o0, range(12, 16), pend0)
        emit_proj_strip(4)
        emit_attn_tiles(0, ps_o0, range(16, 20), pend0)
        emit_proj_strip(5)
        emit_attn_tiles(0, ps_o0, range(20, 24), pend0)
        ps_o_cur, pend_cur = ps_o0, pend0
        for sq in range(1, NSQ):
            ps_o_nxt = [pso.tile([D + 1, SQ], F32, tag=f"ps_o{h}",
                                 name=f"ps_o{h}_{sq}") for h in range(HPC)]
            pend_nxt = []
            emit_attn_tiles(sq, ps_o_nxt, range(0, 4), pend_nxt)
            for args in pend_cur:
                emit_pv(ps_o_cur, *args)
            emit_epilogue(sq - 1, ps_o_cur)
            emit_attn_tiles(sq, ps_o_nxt, range(4, NT), pend_nxt)
            ps_o_cur, pend_cur = ps_o_nxt, pend_nxt
        for args in pend_cur:
            emit_pv(ps_o_cur, *args)
        emit_epilogue(NSQ - 1, ps_o_cur)

    nc.compile()
    return nc


_PROGRAM = None
LAST_RESULTS = None


def _get_program():
    global _PROGRAM
    if _PROGRAM is None:
        _PROGRAM = build_program()
    return _PROGRAM


def _softplus(x):
    x = np.asarray(x, np.float32)
    return np.logaddexp(0.0, x).astype(np.float32)


def host_prep(inputs):
    x = np.asarray(inputs["x"], np.float32)
    edge_index = np.asarray(inputs["edge_index"])
    edge_type = np.asarray(inputs["edge_type"])
    etw = np.asarray(inputs["edge_type_weights"], np.float32)

    def f32(k):
        return np.asarray(inputs[k], np.float32)

    # compose the two linear layers: q2 = x @ (wiq@wq).T + (wiq@bq + biq)
    WQ = f32("wiq") @ f32("wq")
    bQ = f32("wiq") @ f32("bq") + f32("biq")
    WK = f32("wik") @ f32("wk")
    bK = f32("wik") @ f32("bk") + f32("bik")
    WV = f32("wiv") @ f32("wv")
    bV = f32("wiv") @ f32("bv") + f32("biv")
    wo = f32("wo")
    bo = f32("bo")

    # multiplicative mask, transposed: maskT[m, n] = exp(add_mask[n, m])
    w = _softplus(etw)
    NEG = np.float32(-60000.0)
    M = np.full((N, N), NEG, dtype=np.float32)
    src, dst = edge_index[0], edge_index[1]
    wv8 = (w * np.float32(1.0 / SCALE)).astype(np.float32)
    M[src, dst] = wv8[edge_type - 1]           # last write wins, like jax .at[].set
    diag = np.diagonal(M).copy()
    didx = np.arange(N)
    M[didx, didx] = np.where(diag == NEG, wv8[3], diag)
    MT = np.ascontiguousarray(M.T)             # [key m, query n], additive * 8
    # even key tiles use the additive form on the PE, odd tiles the
    # multiplicative exp-form on the DVE
    MT4 = MT.reshape(NT, P, N)
    maskA = MT4[0::2].reshape(N // 2, N).astype(np.float16)
    if MASK_ALL_PE:
        maskM = MT4[1::2].reshape(N // 2, N).astype(np.float16)
    else:
        maskM = np.exp(MT4[1::2].reshape(N // 2, N).astype(np.float64)
                       * np.float64(SCALE)).astype(np.float16)

    xT = np.ascontiguousarray(x.T)

    in_maps = []
    for c in range(NCORES):
        rs = slice(c * CW, (c + 1) * CW)
        in_maps.append({
            "xT": xT,
            "maskA": maskA,
            "maskM": maskM,
            "wqT": np.ascontiguousarray(WQ[rs].T),
            "wkT": np.ascontiguousarray(WK[rs].T),
            "wvT": np.ascontiguousarray(WV[rs].T),
            "bq": np.ascontiguousarray(bQ[rs]),
            "bk": np.ascontiguousarray(bK[rs]),
            "bv": np.ascontiguousarray(bV[rs]),
            "woT": np.ascontiguousarray(wo[:, rs].T),
        })
    return in_maps, bo


def kernel(**inputs) -> np.ndarray:
    global LAST_RESULTS
    in_maps, bo = host_prep(inputs)
    nc = _get_program()
    trace = bool(os.environ.get("KERNEL_TRACE"))
    res = run_bass_kernel_spmd(nc, in_maps, list(range(NCORES)), trace=trace)
    LAST_RESULTS = res
    y = bo[None, :].astype(np.float32).repeat(N, axis=0)
    for c in range(NCORES):
        y += res.results[c]["ypart"]
    return y
